# revision 17
# baseline (speedup 1.0000x reference)
"""Bidirectional-LSTM basecaller on 8 Trainium2 NeuronCores (self-contained).

Layout (HW time, concourse cost model): conv 0.273 ms + lstm 0.494 ms
= 0.767 ms total (baseline for this problem: 12.65 ms).

Launch 1 "conv" (8 cores, SPMD over batch x direction): conv front-end +
  zx = enc@Wx (cores 0-3: forward batch rows; 4-7: time-reversed rows with
  tap-flipped conv kernels -- exact for full reversal).  All matmul operands
  bf16 (1 cycle/row vs 4 for fp32).  ReLUs balanced across ACT and DVE
  (DVE tensor_scalar max / scalar_tensor_tensor fuses relu+residual-add);
  gate biases NOT added here (folded into the lstm's accumulation).
  zx gate cols [I|2J|F|O], J pre-doubled (tanh j = 2*sigmoid(2j)-1).

Launch 2 "lstm" (8 cores): time-chunked recurrence.  Each direction's
  T=2048 steps split into 16 chunks of 128 + 40 warmup steps (forget-gate
  state decay makes truncated history exact to ~4e-3); 4 chunks (streams)
  per core -> 168 serial steps instead of 2048.  State kept TRANSPOSED
  ([200, 32] packed as [128, 64] tiles: block A = rows 0:128 at cols 0:32,
  block B = rows 128:200 on partitions 0:72 at cols 32:64) so the
  recurrence needs no per-step transpose:
    - z^T via 32 small matmuls/step: per (gate, block) chunk, accumulate
      lhsT = zx ring slot (identity rhs injects zx^T), bias row (ones rhs),
      Wh[0:128] (rhs = hA), Wh[128:200] (rhs = hB);
    - ONE sigmoid over all 256 psum cols -> bf16 gates in SBUF;
    - DVE: p/2 = (sig2j - 0.5)*sigi; cf = c*sigf; c' = 2*(p/2) + cf
      (scalar_tensor_tensor fusions); ACT tanh; h = tanh(c')*sigo in one
      [128, 64] op (garbage B-rows 72:128 bounded + never read);
  Per-step latency ~2.94 us, ACT-engine-bound (sigmoid 398 + tanh 238 ns
  busy per stream-step; 4 streams x 636 = 2544 ns of the period).  hseq
  stores issue from the idle Pool queue (SP.SEQ is saturated by ring
  prefetch DMAs at ~2.9 us SEQ-hold each).  Length masking is folded into zx as i/f gate
  logits = -30 at the reset step (exact to ~1e-12), so steps have no mask
  ops.  h history is stored transposed and unpacked on host.

HW facts this build relies on: matmul start=True zeroes the WHOLE PSUM bank
  -> exactly one start per step's accumulation group (skip_group_check);
  lhsT/rhs/psum base partitions 0; bf16 operands for 1-cycle/row matmuls
  and 4x-mode DVE; zx ring = 3 groups of 8 steps, prefetched 1 group ahead
  (slot's last reader finished a full group earlier).

Host: shard prep, zx chunk/warmup assembly (chunk-0 warmup = reset
  pattern), gather, output reversal, valid-length masking, 400x5 decode.
"""
import numpy as np
from contextlib import ExitStack

import concourse.bass as bass
import concourse.bacc as bacc
import concourse.mybir as mybir
from concourse.tile import TileContext
from concourse.bass_utils import run_bass_kernel_spmd

B, T, H, C = 32, 2048, 200, 256
G4 = 4 * H  # 800
FP32 = mybir.dt.float32
BF16 = mybir.dt.bfloat16
SIG = mybir.ActivationFunctionType.Sigmoid
TANH = mybir.ActivationFunctionType.Tanh
RELU = mybir.ActivationFunctionType.Relu
MULT = mybir.AluOpType.mult
ADD = mybir.AluOpType.add
AMAX = mybir.AluOpType.max
SUB = mybir.AluOpType.subtract

CH = 128    # lstm chunk length (16 chunks per direction)
WARM = 40   # warmup steps per chunk
NSTEP = CH + WARM
HU = 8      # steps per hseq tile / ring group


# ----------------------------------------------------------------------------
# Launch 1: conv front-end + zx precompute. 8 (row, dir) pairs per core.
# ----------------------------------------------------------------------------
def build_conv_zx(n_rows=8, tchunk=256):
    nc = bacc.Bacc()
    sig = nc.declare_dram_parameter("sig", [n_rows, T + 2], BF16, isOutput=False)
    k1 = nc.declare_dram_parameter("k1", [2, C], BF16, isOutput=False)
    k1abT = nc.declare_dram_parameter("k1abT", [128, 2], FP32, isOutput=False)
    k2 = nc.declare_dram_parameter("k2", [3, C, C], BF16, isOutput=False)
    k3 = nc.declare_dram_parameter("k3", [C, C], BF16, isOutput=False)
    wx = nc.declare_dram_parameter("wx", [C, G4], BF16, isOutput=False)
    zx = nc.declare_dram_parameter("zx", [n_rows, T, G4], BF16, isOutput=True)

    n_tc = T // tchunk
    with TileContext(nc) as tc:
        with ExitStack() as ctx:
            wpool = ctx.enter_context(tc.tile_pool(name="w", bufs=1))
            spool = ctx.enter_context(tc.tile_pool(name="s", bufs=4))
            c1pool = ctx.enter_context(tc.tile_pool(name="c1", bufs=3))
            c2pool = ctx.enter_context(tc.tile_pool(name="c2", bufs=3))
            epool = ctx.enter_context(tc.tile_pool(name="enc", bufs=3))
            zpool = ctx.enter_context(tc.tile_pool(name="zs", bufs=4))
            ppool = ctx.enter_context(tc.tile_pool(name="ps", bufs=2, space="PSUM"))
            p2pool = ctx.enter_context(tc.tile_pool(name="ps2", bufs=2, space="PSUM"))

            k1_t = wpool.tile([1, 2 * C], BF16)
            k1b_t = wpool.tile([128, 2], FP32)
            k2_t = [wpool.tile([128, 3 * C], BF16, tag=f"k2_{cs}", name=f"k2_{cs}")
                    for cs in range(2)]
            k3_t = [wpool.tile([128, C], BF16, tag=f"k3_{cs}", name=f"k3_{cs}")
                    for cs in range(2)]
            wx_t = [wpool.tile([128, G4], BF16, tag=f"wx_{cs}", name=f"wx_{cs}")
                    for cs in range(2)]
            nc.sync.dma_start(out=k1_t[:, 0:C], in_=k1[0:1, :])
            nc.sync.dma_start(out=k1_t[:, C:2 * C], in_=k1[1:2, :])
            nc.sync.dma_start(out=k1b_t[:], in_=k1abT[:])
            for cs in range(2):
                nc.sync.dma_start(
                    out=k2_t[cs][:].rearrange("p (k c) -> p k c", k=3),
                    in_=k2[:, cs * 128:(cs + 1) * 128, :].transpose([1, 0, 2]))
                nc.sync.dma_start(out=k3_t[cs][:],
                                  in_=k3[cs * 128:(cs + 1) * 128, :])
                nc.sync.dma_start(out=wx_t[cs][:],
                                  in_=wx[cs * 128:(cs + 1) * 128, :])

            TC2 = tchunk + 2
            for r in range(n_rows):
                srow = spool.tile([1, T + 2], BF16, tag="srow")
                nc.sync.dma_start(out=srow[:], in_=sig[r:r + 1, :])
                for ci in range(n_tc):
                    t0 = ci * tchunk
                    st = srow[:, t0:t0 + TC2]
                    c1t = c1pool.tile([128, 2 * TC2], BF16, tag="c1")
                    c1at = c1pool.tile([128, 2 * tchunk], BF16, tag="c1a")
                    for cs in range(2):
                        ps = ppool.tile([128, TC2], FP32, tag="pa", bufs=1)
                        nc.tensor.matmul(
                            ps[:], k1_t[:, cs * 128:(cs + 1) * 128], st[:],
                            start=True, stop=True)
                        nc.vector.tensor_scalar_max(
                            c1t[:, cs * TC2:(cs + 1) * TC2], ps[:], 0.0)
                        ps2 = ppool.tile([128, tchunk], FP32, tag="pb", bufs=1)
                        nc.tensor.matmul(
                            ps2[:], k1_t[:, C + cs * 128:C + (cs + 1) * 128],
                            st[:, 1:tchunk + 1], start=True, stop=True)
                        nc.scalar.activation(
                            c1at[:, cs * tchunk:(cs + 1) * tchunk], ps2[:],
                            RELU, bias=k1b_t[:, cs:cs + 1])
                    c2t = c2pool.tile([128, 2 * tchunk], BF16)
                    for co in range(2):
                        ps = p2pool.tile([128, tchunk], FP32, tag="pc")
                        first = True
                        for k in range(3):
                            for cs in range(2):
                                nc.tensor.matmul(
                                    ps[:],
                                    k2_t[cs][:, k * C + co * 128:
                                             k * C + (co + 1) * 128],
                                    c1t[:, cs * TC2 + k:cs * TC2 + k + tchunk],
                                    start=first, stop=(k == 2 and cs == 1))
                                first = False
                        nc.vector.tensor_scalar_max(
                            c2t[:, co * tchunk:(co + 1) * tchunk], ps[:], 0.0)
                    et = epool.tile([128, 2 * tchunk], BF16)
                    for co in range(2):
                        ps = p2pool.tile([128, tchunk], FP32, tag="pd")
                        for cs in range(2):
                            nc.tensor.matmul(
                                ps[:],
                                k3_t[cs][:, co * 128:(co + 1) * 128],
                                c2t[:, cs * tchunk:(cs + 1) * tchunk],
                                start=(cs == 0), stop=(cs == 1))
                        nc.vector.scalar_tensor_tensor(
                            et[:, co * tchunk:(co + 1) * tchunk], ps[:], 0.0,
                            c1at[:, co * tchunk:(co + 1) * tchunk], AMAX, ADD)
                    for tt in range(tchunk // 128):
                        zs = zpool.tile([128, G4], BF16)
                        for half in range(2):
                            ps = p2pool.tile([128, 400], FP32, tag="pe")
                            for cs in range(2):
                                nc.tensor.matmul(
                                    ps[:],
                                    et[:, cs * tchunk + tt * 128:
                                       cs * tchunk + tt * 128 + 128],
                                    wx_t[cs][:, half * 400:(half + 1) * 400],
                                    start=(cs == 0), stop=(cs == 1))
                            if half == 0:
                                nc.vector.tensor_copy(zs[:, 0:400], ps[:])
                            else:
                                nc.scalar.copy(zs[:, 400:800], ps[:])
                        nc.sync.dma_start(
                            out=zx[r, t0 + tt * 128:t0 + (tt + 1) * 128, :],
                            in_=zs[:])
    nc.compile()
    return nc


# ----------------------------------------------------------------------------
# Launch 2: chunked LSTM recurrence, transposed state, 2 streams per core.
# ----------------------------------------------------------------------------
def build_lstm2(n_streams=4, n_steps=NSTEP, U=24):
    """zxin[s]: [32, n_steps, 800] bf16, gate cols [I|2J|F|O], bias folded,
    length-reset encoded as i/f=-30 cols.  hseqT[s]: [128, n_steps*64] bf16,
    h(t) packed-transposed at col t*64 (cols 0:32 = h rows 0:128; cols 32:64
    partitions 0:72 = h rows 128:200)."""
    nc = bacc.Bacc()
    w0 = nc.declare_dram_parameter("w0", [128, G4], BF16, isOutput=False)
    w1 = nc.declare_dram_parameter("w1", [72, G4], BF16, isOutput=False)
    bT = nc.declare_dram_parameter("bT", [1, G4], BF16, isOutput=False)
    id32 = nc.declare_dram_parameter("id32", [32, 32], BF16, isOutput=False)
    zxin = nc.declare_dram_parameter("zxin", [n_streams, 32, n_steps, G4],
                                     BF16, isOutput=False)
    hseqT = nc.declare_dram_parameter("hseqT", [n_streams, 128, n_steps * 64],
                                      BF16, isOutput=True)

    n_grp = n_steps // HU
    assert n_steps % HU == 0 and U % HU == 0

    with TileContext(nc) as tc:
        with ExitStack() as ctx:
            wpool = ctx.enter_context(tc.tile_pool(name="w", bufs=1))
            rpool = ctx.enter_context(tc.tile_pool(name="ring", bufs=1))
            spool = ctx.enter_context(tc.tile_pool(name="st", bufs=1))
            hpool = ctx.enter_context(tc.tile_pool(name="hst", bufs=3))
            gpool = ctx.enter_context(tc.tile_pool(name="g", bufs=3))
            tpool = ctx.enter_context(tc.tile_pool(name="tmp", bufs=2))
            zpsp = ctx.enter_context(tc.tile_pool(name="zps", bufs=2,
                                                  space="PSUM"))

            w0t = wpool.tile([128, G4], BF16)
            w1t = wpool.tile([72, G4], BF16)
            i32t = wpool.tile([32, 32], BF16)
            bTt = wpool.tile([1, G4], BF16)
            ones32 = wpool.tile([1, 32], BF16)
            nc.sync.dma_start(out=w0t[:], in_=w0[:])
            nc.sync.dma_start(out=w1t[:], in_=w1[:])
            nc.sync.dma_start(out=i32t[:], in_=id32[:])
            nc.sync.dma_start(out=bTt[:], in_=bT[:])
            nc.vector.memset(ones32[:], 1.0)

            rings, cts, h0s = [], [], []
            for s in range(n_streams):
                ring = rpool.tile([32, U * G4], BF16, tag=f"ring{s}",
                                  name=f"ring{s}")
                ct = spool.tile([128, 64], BF16, tag=f"ct{s}", name=f"ct{s}")
                h0 = spool.tile([128, 64], BF16, tag=f"h0{s}", name=f"h0{s}")
                nc.vector.memset(ct[:], 0.0)
                nc.vector.memset(h0[:], 0.0)
                # preload ring group 0 only; rest prefetched 1 group ahead
                nc.sync.dma_start(
                    out=ring[:, 0:HU * G4
                             ].rearrange("p (s g) -> p s g", s=HU),
                    in_=zxin[s, :, 0:HU, :])
                rings.append(ring)
                cts.append(ct)
                h0s.append(h0)

            # chunk table: (psum col, zx/w col, K-size)
            chunks = []
            for g4 in range(4):
                chunks.append((g4 * 64, g4 * 200, 128))           # block A
                chunks.append((g4 * 64 + 32, g4 * 200 + 128, 72))  # block B

            def step(s, u, hst, hprev):
                slot = (u % U) * G4
                zp = zpsp.tile([128, 256], FP32, tag=f"zp{s}")
                # start=True zeroes the WHOLE PSUM bank -> exactly one start
                # (first zx matmul); everything else accumulates in place.
                first = True
                for co, gc, csz in chunks:
                    nc.tensor.matmul(
                        zp[0:csz, co:co + 32],
                        rings[s][:, slot + gc:slot + gc + csz],
                        i32t[:], start=first, stop=False,
                        skip_group_check=True)
                    first = False
                for co, gc, csz in chunks:
                    nc.tensor.matmul(
                        zp[0:csz, co:co + 32], bTt[:, gc:gc + csz],
                        ones32[:], start=False, stop=False,
                        skip_group_check=True)
                hA = hprev[:, 0:32]
                hB = hprev[0:72, 32:64]
                for idx, (co, gc, csz) in enumerate(chunks):
                    nc.tensor.matmul(zp[0:csz, co:co + 32],
                                     w0t[:, gc:gc + csz], hA,
                                     start=False, stop=False,
                                     skip_group_check=True)
                    nc.tensor.matmul(zp[0:csz, co:co + 32],
                                     w1t[:, gc:gc + csz], hB,
                                     start=False, stop=(idx == 7),
                                     skip_group_check=True)
                g = gpool.tile([128, 256], BF16, tag=f"g{s}")
                nc.scalar.activation(g[:], zp[:], SIG)
                # p/2 = (sig(2j) - 0.5) * sig(i)  [tanh j = 2 sig(2j) - 1]
                ph = tpool.tile([128, 64], BF16, tag=f"ph{s}")
                nc.vector.scalar_tensor_tensor(ph[:], g[:, 64:128], 0.5,
                                               g[:, 0:64], SUB, MULT)
                cf = tpool.tile([128, 64], BF16, tag=f"cf{s}")
                nc.vector.tensor_mul(cf[:], cts[s][:], g[:, 128:192])
                # c = 2*(p/2) + cf
                nc.vector.scalar_tensor_tensor(cts[s][:], ph[:], 2.0,
                                               cf[:], MULT, ADD)
                th = tpool.tile([128, 64], BF16, tag=f"th{s}")
                nc.scalar.activation(th[:], cts[s][:], TANH)
                o = (u % HU) * 64
                # one op; partitions 72:128 of the B half are garbage but
                # bounded (psum zeroed by start=True) and never read
                nc.vector.tensor_mul(hst[:, o:o + 64], th[:, 0:64],
                                     g[:, 192:256])

            hsts = [None] * n_streams
            for grp in range(n_grp):
                cur = []
                for s in range(n_streams):
                    # prefetch ring group grp+1 into its slot (ring holds
                    # U//HU=3 groups; the slot's last reader was grp-2, a
                    # full group ago -> safe even if lhsT WAR is untracked)
                    pg = grp + 1
                    if pg < n_grp:
                        half = (pg % (U // HU)) * HU
                        nc.sync.dma_start(
                            out=rings[s][:, half * G4:(half + HU) * G4
                                         ].rearrange("p (s g) -> p s g", s=HU),
                            in_=zxin[s, :, pg * HU:(pg + 1) * HU, :])
                    hst = hpool.tile([128, HU * 64], BF16, tag=f"hst{s}",
                                     name=f"hst{s}")
                    cur.append(hst)
                for s in range(n_streams):
                    for k in range(HU):
                        u = grp * HU + k
                        if u == 0:
                            hprev = h0s[s][:]
                        elif k == 0:
                            hprev = hsts[s][:, (HU - 1) * 64:HU * 64]
                        else:
                            hprev = cur[s][:, (k - 1) * 64:k * 64]
                        step(s, u, cur[s][:], hprev)
                for s in range(n_streams):
                    # issue from the otherwise-idle Pool queue: SP.SEQ is
                    # saturated by ring prefetches (~2.9us hold per DMA)
                    nc.gpsimd.dma_start(
                        out=hseqT[s, :, grp * HU * 64:(grp + 1) * HU * 64],
                        in_=cur[s][:])
                    hsts[s] = cur[s]
    nc.compile()
    return nc


# ----------------------------------------------------------------------------
# host-side runners
# ----------------------------------------------------------------------------
_NC_CACHE = {}
LAUNCH_WALLS = {}


def run_conv_zx(in_maps, **kw):
    import time
    if "conv" not in _NC_CACHE:
        _NC_CACHE["conv"] = build_conv_zx()
    nc = _NC_CACHE["conv"]
    t0 = time.time()
    res = run_bass_kernel_spmd(nc, in_maps, list(range(len(in_maps))), **kw)
    out = [r["zx"] for r in res.results]
    LAUNCH_WALLS["conv"] = time.time() - t0
    return out, res


def run_lstm(in_maps, **kw):
    import time
    if "lstm" not in _NC_CACHE:
        _NC_CACHE["lstm"] = build_lstm2()
    nc = _NC_CACHE["lstm"]
    t0 = time.time()
    res = run_bass_kernel_spmd(nc, in_maps, list(range(len(in_maps))), **kw)
    out = [r["hseqT"] for r in res.results]
    LAUNCH_WALLS["lstm"] = time.time() - t0
    return out, res


def _bf16(x):
    import ml_dtypes
    return np.asarray(x).astype(ml_dtypes.bfloat16)


def _perm_cols(w):
    """reference gate order [i, j, f, o] -> [I | 2*J | F | O] (800 cols)."""
    i, j, f, o = (w[..., k * H:(k + 1) * H] for k in range(4))
    return np.concatenate([i, 2.0 * j, f, o], axis=-1)


def _perm_bias(b):
    i, j, f, o = (b[k * H:(k + 1) * H] for k in range(4))
    return np.concatenate([i, 2.0 * j, f + 1.0, o], axis=-1)


def _unpack_hseqT(arr, n_steps):
    """[128, n_steps*64] bf16 -> [32, n_steps, 200] fp32"""
    a = np.asarray(arr, np.float32).reshape(128, n_steps, 2, 32)
    out = np.empty((32, n_steps, 200), np.float32)
    out[:, :, 0:128] = a[:, :, 0, :].transpose(2, 1, 0)
    out[:, :, 128:200] = a[0:72, :, 1, :].transpose(2, 1, 0)
    return out


def kernel(signals, sig_length, k1w, k1aw, k1ab, k2w, k3w, Wf, bf, Wb, bb,
           Wd, bd):
    import ml_dtypes
    sig = np.ascontiguousarray(np.asarray(signals, np.float32)[:, :, 0])
    L = np.asarray(sig_length).astype(np.int64)
    k1 = np.stack([np.asarray(k1w, np.float32)[0, 0],
                   np.asarray(k1aw, np.float32)[0, 0]])  # [2, C]
    k1abT = np.ascontiguousarray(
        np.asarray(k1ab, np.float32).reshape(2, 128).T)  # [128, 2]
    k2w = np.asarray(k2w, np.float32)
    k3 = np.ascontiguousarray(np.asarray(k3w, np.float32)[0])
    Wf = np.asarray(Wf, np.float32); Wb = np.asarray(Wb, np.float32)
    bfp = _perm_bias(np.asarray(bf, np.float32))
    bbp = _perm_bias(np.asarray(bb, np.float32))
    Wd = np.asarray(Wd, np.float32); bd = np.asarray(bd, np.float32)

    Wxf = _perm_cols(Wf[:C]); Whf = _perm_cols(Wf[C:])
    Wxb = _perm_cols(Wb[:C]); Whb_ = _perm_cols(Wb[C:])

    # ---------------- launch 1: conv + zx ----------------
    sig_rev = np.ascontiguousarray(sig[:, ::-1])
    k2_flip = np.ascontiguousarray(k2w[::-1])
    sig_p = np.pad(sig, ((0, 0), (1, 1)))
    sig_rp = np.pad(sig_rev, ((0, 0), (1, 1)))
    in_maps = []
    for g in range(4):
        in_maps.append(dict(sig=_bf16(sig_p[8 * g:8 * g + 8]), k1=_bf16(k1),
                            k1abT=k1abT, k2=_bf16(k2w), k3=_bf16(k3),
                            wx=_bf16(Wxf)))
    for g in range(4):
        in_maps.append(dict(sig=_bf16(sig_rp[8 * g:8 * g + 8]), k1=_bf16(k1),
                            k1abT=k1abT, k2=_bf16(k2_flip), k3=_bf16(k3),
                            wx=_bf16(Wxb)))
    zx_list, _ = run_conv_zx(in_maps)

    # zx_f/zx_b: [32, T, 800] bf16 (bw rows are fully time-reversed)
    zx_f = np.concatenate([np.asarray(z) for z in zx_list[0:4]], axis=0)
    zx_b = np.concatenate([np.asarray(z) for z in zx_list[4:8]], axis=0)

    # length reset for bw: zero state entering scan step T-L by forcing
    # i/f gate logits to -30 at step T-L-1 (c_new ~ 0, h_new ~ 0).
    NEG = ml_dtypes.bfloat16(-30.0)
    for b in range(B):
        tr = T - int(L[b]) - 1
        if 0 <= tr < T:
            zx_b[b, tr, 0:H] = NEG
            zx_b[b, tr, 2 * H:3 * H] = NEG

    # per-stream zx assembly: chunk k covers steps [k*CH, (k+1)*CH) with
    # WARM warmup steps before; chunk 0's warmup is the reset pattern.
    reset_blk = np.zeros((B, WARM, G4), ml_dtypes.bfloat16)
    reset_blk[:, :, 0:H] = NEG
    reset_blk[:, :, 2 * H:3 * H] = NEG

    def stream_zx(zx_full, k):
        t0 = k * CH
        if t0 == 0:
            return np.concatenate([reset_blk, zx_full[:, 0:CH]], axis=1)
        return zx_full[:, t0 - WARM:t0 + CH]

    # ---------------- launch 2: recurrence ----------------
    id32 = np.eye(32, dtype=np.float32)
    in_maps2 = []
    for c in range(8):
        if c < 4:
            zxd, wh, bp = zx_f, Whf, bfp
        else:
            zxd, wh, bp = zx_b, Whb_, bbp
        k0 = 4 * (c % 4)
        zxin = np.stack([stream_zx(zxd, k0 + s) for s in range(4)], axis=0)
        in_maps2.append(dict(w0=_bf16(wh[0:128]), w1=_bf16(wh[128:200]),
                             bT=_bf16(bp[None, :]), id32=_bf16(id32),
                             zxin=zxin))
    hseqs, _ = run_lstm(in_maps2)

    # ---------------- host decode ----------------
    fw = np.empty((B, T, H), np.float32)
    bw_s = np.empty((B, T, H), np.float32)
    for c in range(8):
        hs = np.asarray(hseqs[c])
        dst = fw if c < 4 else bw_s
        for s in range(4):
            k = 4 * (c % 4) + s
            h = _unpack_hseqT(hs[s], NSTEP)[:, WARM:]
            dst[:, k * CH:(k + 1) * CH] = h
    bw = bw_s[:, ::-1, :]                                      # t = T-1-s
    bi = np.concatenate([fw, bw], axis=-1)                     # [32, T, 2H]
    logits = bi.reshape(-1, 2 * H) @ Wd + bd
    logits = logits.reshape(B, T, 5).astype(np.float32)
    tmask = np.arange(T)[None, :] >= L[:, None]
    logits[tmask] = bd
    return logits


if __name__ == "__main__":
    import jax, reference
    cpu = jax.devices("cpu")[0]
    with jax.default_device(cpu):
        inputs = {k: np.asarray(v) for k, v in reference.setup_inputs().items()}
        expected = np.asarray(jax.jit(reference.reference, backend="cpu")(
            **{k: jax.device_put(v, cpu) for k, v in inputs.items()}))
    actual = kernel(**inputs)
    err = np.abs(actual - expected).max() / (np.abs(expected).max() + 1e-9)
    print("Relative error:", err)


# revision 18
# speedup vs baseline: 1.0068x; 1.0068x over previous
"""Bidirectional-LSTM basecaller on 8 Trainium2 NeuronCores (self-contained).

Layout (HW time, concourse cost model): conv 0.273 ms + lstm 0.494 ms
= 0.767 ms total (baseline for this problem: 12.65 ms).

Launch 1 "conv" (8 cores, SPMD over batch x direction): conv front-end +
  zx = enc@Wx (cores 0-3: forward batch rows; 4-7: time-reversed rows with
  tap-flipped conv kernels -- exact for full reversal).  All matmul operands
  bf16 (1 cycle/row vs 4 for fp32).  ReLUs balanced across ACT and DVE
  (DVE tensor_scalar max / scalar_tensor_tensor fuses relu+residual-add);
  gate biases NOT added here (folded into the lstm's accumulation).
  zx gate cols [I|2J|F|O], J pre-doubled (tanh j = 2*sigmoid(2j)-1).

Launch 2 "lstm" (8 cores): time-chunked recurrence.  Each direction's
  T=2048 steps split into 16 chunks of 128 + 40 warmup steps (forget-gate
  state decay makes truncated history exact to ~4e-3); 4 chunks (streams)
  per core -> 168 serial steps instead of 2048.  State kept TRANSPOSED
  ([200, 32] packed as [128, 64] tiles: block A = rows 0:128 at cols 0:32,
  block B = rows 128:200 on partitions 0:72 at cols 32:64) so the
  recurrence needs no per-step transpose:
    - z^T via 32 small matmuls/step: per (gate, block) chunk, accumulate
      lhsT = zx ring slot (identity rhs injects zx^T), bias row (ones rhs),
      Wh[0:128] (rhs = hA), Wh[128:200] (rhs = hB);
    - ONE sigmoid over all 256 psum cols -> bf16 gates in SBUF;
    - DVE: p/2 = (sig2j - 0.5)*sigi; cf = c*sigf; c' = 2*(p/2) + cf
      (scalar_tensor_tensor fusions); ACT tanh; h = tanh(c')*sigo in one
      [128, 64] op (garbage B-rows 72:128 bounded + never read);
  Per-step latency ~2.94 us, ACT-engine-bound (sigmoid 398 + tanh 238 ns
  busy per stream-step; 4 streams x 636 = 2544 ns of the period).  hseq
  stores issue from the idle Pool queue (SP.SEQ is saturated by ring
  prefetch DMAs at ~2.9 us SEQ-hold each).  Length masking is folded into zx as i/f gate
  logits = -30 at the reset step (exact to ~1e-12), so steps have no mask
  ops.  h history is stored transposed and unpacked on host.

HW facts this build relies on: matmul start=True zeroes the WHOLE PSUM bank
  -> exactly one start per step's accumulation group (skip_group_check);
  lhsT/rhs/psum base partitions 0; bf16 operands for 1-cycle/row matmuls
  and 4x-mode DVE; zx ring = 3 groups of 8 steps, prefetched 1 group ahead
  (slot's last reader finished a full group earlier).

Host: shard prep, zx chunk/warmup assembly (chunk-0 warmup = reset
  pattern), gather, output reversal, valid-length masking, 400x5 decode.
"""
import numpy as np
from contextlib import ExitStack

import concourse.bass as bass
import concourse.bacc as bacc
import concourse.mybir as mybir
from concourse.tile import TileContext
from concourse.bass_utils import run_bass_kernel_spmd

B, T, H, C = 32, 2048, 200, 256
G4 = 4 * H  # 800
FP32 = mybir.dt.float32
BF16 = mybir.dt.bfloat16
SIG = mybir.ActivationFunctionType.Sigmoid
TANH = mybir.ActivationFunctionType.Tanh
RELU = mybir.ActivationFunctionType.Relu
MULT = mybir.AluOpType.mult
ADD = mybir.AluOpType.add
AMAX = mybir.AluOpType.max
SUB = mybir.AluOpType.subtract

CH = 128    # lstm chunk length (16 chunks per direction)
WARM = 40   # warmup steps per chunk
NSTEP = CH + WARM
HU = 8      # steps per hseq tile / ring group


# ----------------------------------------------------------------------------
# Launch 1: conv front-end + zx precompute. 8 (row, dir) pairs per core.
# ----------------------------------------------------------------------------
def build_conv_zx(n_rows=8, tchunk=256):
    nc = bacc.Bacc()
    sig = nc.declare_dram_parameter("sig", [n_rows, T + 2], BF16, isOutput=False)
    k1 = nc.declare_dram_parameter("k1", [2, C], BF16, isOutput=False)
    k1abT = nc.declare_dram_parameter("k1abT", [128, 2], FP32, isOutput=False)
    k2 = nc.declare_dram_parameter("k2", [3, C, C], BF16, isOutput=False)
    k3 = nc.declare_dram_parameter("k3", [C, C], BF16, isOutput=False)
    wx = nc.declare_dram_parameter("wx", [C, G4], BF16, isOutput=False)
    zx = nc.declare_dram_parameter("zx", [n_rows, T, G4], BF16, isOutput=True)

    n_tc = T // tchunk
    with TileContext(nc) as tc:
        with ExitStack() as ctx:
            wpool = ctx.enter_context(tc.tile_pool(name="w", bufs=1))
            spool = ctx.enter_context(tc.tile_pool(name="s", bufs=4))
            c1pool = ctx.enter_context(tc.tile_pool(name="c1", bufs=3))
            c2pool = ctx.enter_context(tc.tile_pool(name="c2", bufs=3))
            epool = ctx.enter_context(tc.tile_pool(name="enc", bufs=3))
            zpool = ctx.enter_context(tc.tile_pool(name="zs", bufs=4))
            ppool = ctx.enter_context(tc.tile_pool(name="ps", bufs=2, space="PSUM"))
            p2pool = ctx.enter_context(tc.tile_pool(name="ps2", bufs=2, space="PSUM"))

            k1_t = wpool.tile([1, 2 * C], BF16)
            k1b_t = wpool.tile([128, 2], FP32)
            k2_t = [wpool.tile([128, 3 * C], BF16, tag=f"k2_{cs}", name=f"k2_{cs}")
                    for cs in range(2)]
            k3_t = [wpool.tile([128, C], BF16, tag=f"k3_{cs}", name=f"k3_{cs}")
                    for cs in range(2)]
            wx_t = [wpool.tile([128, G4], BF16, tag=f"wx_{cs}", name=f"wx_{cs}")
                    for cs in range(2)]
            nc.sync.dma_start(out=k1_t[:, 0:C], in_=k1[0:1, :])
            nc.sync.dma_start(out=k1_t[:, C:2 * C], in_=k1[1:2, :])
            nc.sync.dma_start(out=k1b_t[:], in_=k1abT[:])
            for cs in range(2):
                nc.sync.dma_start(
                    out=k2_t[cs][:].rearrange("p (k c) -> p k c", k=3),
                    in_=k2[:, cs * 128:(cs + 1) * 128, :].transpose([1, 0, 2]))
                nc.sync.dma_start(out=k3_t[cs][:],
                                  in_=k3[cs * 128:(cs + 1) * 128, :])
                nc.sync.dma_start(out=wx_t[cs][:],
                                  in_=wx[cs * 128:(cs + 1) * 128, :])

            TC2 = tchunk + 2
            for r in range(n_rows):
                srow = spool.tile([1, T + 2], BF16, tag="srow")
                nc.sync.dma_start(out=srow[:], in_=sig[r:r + 1, :])
                for ci in range(n_tc):
                    t0 = ci * tchunk
                    st = srow[:, t0:t0 + TC2]
                    c1t = c1pool.tile([128, 2 * TC2], BF16, tag="c1")
                    c1at = c1pool.tile([128, 2 * tchunk], BF16, tag="c1a")
                    for cs in range(2):
                        ps = ppool.tile([128, TC2], FP32, tag="pa", bufs=1)
                        nc.tensor.matmul(
                            ps[:], k1_t[:, cs * 128:(cs + 1) * 128], st[:],
                            start=True, stop=True)
                        nc.vector.tensor_scalar_max(
                            c1t[:, cs * TC2:(cs + 1) * TC2], ps[:], 0.0)
                        ps2 = ppool.tile([128, tchunk], FP32, tag="pb", bufs=1)
                        nc.tensor.matmul(
                            ps2[:], k1_t[:, C + cs * 128:C + (cs + 1) * 128],
                            st[:, 1:tchunk + 1], start=True, stop=True)
                        nc.scalar.activation(
                            c1at[:, cs * tchunk:(cs + 1) * tchunk], ps2[:],
                            RELU, bias=k1b_t[:, cs:cs + 1])
                    c2t = c2pool.tile([128, 2 * tchunk], BF16)
                    for co in range(2):
                        ps = p2pool.tile([128, tchunk], FP32, tag="pc")
                        first = True
                        for k in range(3):
                            for cs in range(2):
                                nc.tensor.matmul(
                                    ps[:],
                                    k2_t[cs][:, k * C + co * 128:
                                             k * C + (co + 1) * 128],
                                    c1t[:, cs * TC2 + k:cs * TC2 + k + tchunk],
                                    start=first, stop=(k == 2 and cs == 1))
                                first = False
                        nc.vector.tensor_scalar_max(
                            c2t[:, co * tchunk:(co + 1) * tchunk], ps[:], 0.0)
                    et = epool.tile([128, 2 * tchunk], BF16)
                    for co in range(2):
                        ps = p2pool.tile([128, tchunk], FP32, tag="pd")
                        for cs in range(2):
                            nc.tensor.matmul(
                                ps[:],
                                k3_t[cs][:, co * 128:(co + 1) * 128],
                                c2t[:, cs * tchunk:(cs + 1) * tchunk],
                                start=(cs == 0), stop=(cs == 1))
                        nc.vector.scalar_tensor_tensor(
                            et[:, co * tchunk:(co + 1) * tchunk], ps[:], 0.0,
                            c1at[:, co * tchunk:(co + 1) * tchunk], AMAX, ADD)
                    for tt in range(tchunk // 128):
                        zs = zpool.tile([128, G4], BF16)
                        for half in range(2):
                            ps = p2pool.tile([128, 400], FP32, tag="pe")
                            for cs in range(2):
                                nc.tensor.matmul(
                                    ps[:],
                                    et[:, cs * tchunk + tt * 128:
                                       cs * tchunk + tt * 128 + 128],
                                    wx_t[cs][:, half * 400:(half + 1) * 400],
                                    start=(cs == 0), stop=(cs == 1))
                            if half == 0:
                                nc.vector.tensor_copy(zs[:, 0:400], ps[:])
                            else:
                                nc.scalar.copy(zs[:, 400:800], ps[:])
                        nc.sync.dma_start(
                            out=zx[r, t0 + tt * 128:t0 + (tt + 1) * 128, :],
                            in_=zs[:])
    nc.compile()
    return nc


# ----------------------------------------------------------------------------
# Launch 2: chunked LSTM recurrence, transposed state, 2 streams per core.
# ----------------------------------------------------------------------------
def build_lstm2(n_streams=4, n_steps=NSTEP, U=24):
    """zxin[s]: [32, n_steps, 800] bf16, gate cols [I|2J|F|O], bias folded,
    length-reset encoded as i/f=-30 cols.  hseqT[s]: [128, n_steps*64] bf16,
    h(t) packed-transposed at col t*64 (cols 0:32 = h rows 0:128; cols 32:64
    partitions 0:72 = h rows 128:200)."""
    nc = bacc.Bacc()
    w0 = nc.declare_dram_parameter("w0", [128, G4], BF16, isOutput=False)
    w1 = nc.declare_dram_parameter("w1", [72, G4], BF16, isOutput=False)
    bT = nc.declare_dram_parameter("bT", [1, G4], BF16, isOutput=False)
    id32 = nc.declare_dram_parameter("id32", [32, 32], BF16, isOutput=False)
    zxin = nc.declare_dram_parameter("zxin", [n_streams, 32, n_steps, G4],
                                     BF16, isOutput=False)
    hseqT = nc.declare_dram_parameter("hseqT", [n_streams, 128, n_steps * 64],
                                      BF16, isOutput=True)

    n_grp = n_steps // HU
    assert n_steps % HU == 0 and U % HU == 0

    with TileContext(nc) as tc:
        with ExitStack() as ctx:
            wpool = ctx.enter_context(tc.tile_pool(name="w", bufs=1))
            rpool = ctx.enter_context(tc.tile_pool(name="ring", bufs=1))
            spool = ctx.enter_context(tc.tile_pool(name="st", bufs=1))
            hpool = ctx.enter_context(tc.tile_pool(name="hst", bufs=3))
            gpool = ctx.enter_context(tc.tile_pool(name="g", bufs=3))
            tpool = ctx.enter_context(tc.tile_pool(name="tmp", bufs=2))
            zpsp = ctx.enter_context(tc.tile_pool(name="zps", bufs=2,
                                                  space="PSUM"))

            w0t = wpool.tile([128, G4], BF16)
            w1t = wpool.tile([72, G4], BF16)
            i32t = wpool.tile([32, 32], BF16)
            bTt = wpool.tile([1, G4], BF16)
            ones32 = wpool.tile([1, 32], BF16)
            nc.sync.dma_start(out=w0t[:], in_=w0[:])
            nc.sync.dma_start(out=w1t[:], in_=w1[:])
            nc.sync.dma_start(out=i32t[:], in_=id32[:])
            nc.sync.dma_start(out=bTt[:], in_=bT[:])
            nc.vector.memset(ones32[:], 1.0)

            rings, cts, h0s = [], [], []
            for s in range(n_streams):
                ring = rpool.tile([32, U * G4], BF16, tag=f"ring{s}",
                                  name=f"ring{s}")
                ct = spool.tile([128, 64], BF16, tag=f"ct{s}", name=f"ct{s}")
                h0 = spool.tile([128, 64], BF16, tag=f"h0{s}", name=f"h0{s}")
                nc.vector.memset(ct[:], 0.0)
                nc.vector.memset(h0[:], 0.0)
                # preload ring group 0 only; rest prefetched 1 group ahead
                # (split across SP/Pool queues: serialized SP issue costs
                # ~3us per DMA at startup)
                eng = nc.gpsimd if s % 2 else nc.sync
                eng.dma_start(
                    out=ring[:, 0:HU * G4
                             ].rearrange("p (s g) -> p s g", s=HU),
                    in_=zxin[s, :, 0:HU, :])
                rings.append(ring)
                cts.append(ct)
                h0s.append(h0)

            # chunk table: (psum col, zx/w col, K-size)
            chunks = []
            for g4 in range(4):
                chunks.append((g4 * 64, g4 * 200, 128))           # block A
                chunks.append((g4 * 64 + 32, g4 * 200 + 128, 72))  # block B

            def step(s, u, hst, hprev):
                slot = (u % U) * G4
                zp = zpsp.tile([128, 256], FP32, tag=f"zp{s}")
                # start=True zeroes the WHOLE PSUM bank -> exactly one start
                # (first zx matmul); everything else accumulates in place.
                first = True
                for co, gc, csz in chunks:
                    nc.tensor.matmul(
                        zp[0:csz, co:co + 32],
                        rings[s][:, slot + gc:slot + gc + csz],
                        i32t[:], start=first, stop=False,
                        skip_group_check=True)
                    first = False
                for co, gc, csz in chunks:
                    nc.tensor.matmul(
                        zp[0:csz, co:co + 32], bTt[:, gc:gc + csz],
                        ones32[:], start=False, stop=False,
                        skip_group_check=True)
                hA = hprev[:, 0:32]
                hB = hprev[0:72, 32:64]
                for idx, (co, gc, csz) in enumerate(chunks):
                    nc.tensor.matmul(zp[0:csz, co:co + 32],
                                     w0t[:, gc:gc + csz], hA,
                                     start=False, stop=False,
                                     skip_group_check=True)
                    nc.tensor.matmul(zp[0:csz, co:co + 32],
                                     w1t[:, gc:gc + csz], hB,
                                     start=False, stop=(idx == 7),
                                     skip_group_check=True)
                g = gpool.tile([128, 256], BF16, tag=f"g{s}")
                nc.scalar.activation(g[:], zp[:], SIG)
                # p/2 = (sig(2j) - 0.5) * sig(i)  [tanh j = 2 sig(2j) - 1]
                ph = tpool.tile([128, 64], BF16, tag=f"ph{s}")
                nc.vector.scalar_tensor_tensor(ph[:], g[:, 64:128], 0.5,
                                               g[:, 0:64], SUB, MULT)
                cf = tpool.tile([128, 64], BF16, tag=f"cf{s}")
                nc.vector.tensor_mul(cf[:], cts[s][:], g[:, 128:192])
                # c = 2*(p/2) + cf
                nc.vector.scalar_tensor_tensor(cts[s][:], ph[:], 2.0,
                                               cf[:], MULT, ADD)
                th = tpool.tile([128, 64], BF16, tag=f"th{s}")
                nc.scalar.activation(th[:], cts[s][:], TANH)
                o = (u % HU) * 64
                # one op; partitions 72:128 of the B half are garbage but
                # bounded (psum zeroed by start=True) and never read
                nc.vector.tensor_mul(hst[:, o:o + 64], th[:, 0:64],
                                     g[:, 192:256])

            hsts = [None] * n_streams
            for grp in range(n_grp):
                cur = []
                for s in range(n_streams):
                    # prefetch ring group grp+1 into its slot (ring holds
                    # U//HU=3 groups; the slot's last reader was grp-2, a
                    # full group ago -> safe even if lhsT WAR is untracked)
                    pg = grp + 1
                    if pg < n_grp:
                        half = (pg % (U // HU)) * HU
                        nc.sync.dma_start(
                            out=rings[s][:, half * G4:(half + HU) * G4
                                         ].rearrange("p (s g) -> p s g", s=HU),
                            in_=zxin[s, :, pg * HU:(pg + 1) * HU, :])
                    hst = hpool.tile([128, HU * 64], BF16, tag=f"hst{s}",
                                     name=f"hst{s}")
                    cur.append(hst)
                for s in range(n_streams):
                    for k in range(HU):
                        u = grp * HU + k
                        if u == 0:
                            hprev = h0s[s][:]
                        elif k == 0:
                            hprev = hsts[s][:, (HU - 1) * 64:HU * 64]
                        else:
                            hprev = cur[s][:, (k - 1) * 64:k * 64]
                        step(s, u, cur[s][:], hprev)
                for s in range(n_streams):
                    # issue from the otherwise-idle Pool queue: SP.SEQ is
                    # saturated by ring prefetches (~2.9us hold per DMA)
                    nc.gpsimd.dma_start(
                        out=hseqT[s, :, grp * HU * 64:(grp + 1) * HU * 64],
                        in_=cur[s][:])
                    hsts[s] = cur[s]
    nc.compile()
    return nc


# ----------------------------------------------------------------------------
# host-side runners
# ----------------------------------------------------------------------------
_NC_CACHE = {}
LAUNCH_WALLS = {}


def run_conv_zx(in_maps, **kw):
    import time
    if "conv" not in _NC_CACHE:
        _NC_CACHE["conv"] = build_conv_zx()
    nc = _NC_CACHE["conv"]
    t0 = time.time()
    res = run_bass_kernel_spmd(nc, in_maps, list(range(len(in_maps))), **kw)
    out = [r["zx"] for r in res.results]
    LAUNCH_WALLS["conv"] = time.time() - t0
    return out, res


def run_lstm(in_maps, **kw):
    import time
    if "lstm" not in _NC_CACHE:
        _NC_CACHE["lstm"] = build_lstm2()
    nc = _NC_CACHE["lstm"]
    t0 = time.time()
    res = run_bass_kernel_spmd(nc, in_maps, list(range(len(in_maps))), **kw)
    out = [r["hseqT"] for r in res.results]
    LAUNCH_WALLS["lstm"] = time.time() - t0
    return out, res


def _bf16(x):
    import ml_dtypes
    return np.asarray(x).astype(ml_dtypes.bfloat16)


def _perm_cols(w):
    """reference gate order [i, j, f, o] -> [I | 2*J | F | O] (800 cols)."""
    i, j, f, o = (w[..., k * H:(k + 1) * H] for k in range(4))
    return np.concatenate([i, 2.0 * j, f, o], axis=-1)


def _perm_bias(b):
    i, j, f, o = (b[k * H:(k + 1) * H] for k in range(4))
    return np.concatenate([i, 2.0 * j, f + 1.0, o], axis=-1)


def _unpack_hseqT(arr, n_steps):
    """[128, n_steps*64] bf16 -> [32, n_steps, 200] fp32"""
    a = np.asarray(arr, np.float32).reshape(128, n_steps, 2, 32)
    out = np.empty((32, n_steps, 200), np.float32)
    out[:, :, 0:128] = a[:, :, 0, :].transpose(2, 1, 0)
    out[:, :, 128:200] = a[0:72, :, 1, :].transpose(2, 1, 0)
    return out


def kernel(signals, sig_length, k1w, k1aw, k1ab, k2w, k3w, Wf, bf, Wb, bb,
           Wd, bd):
    import ml_dtypes
    sig = np.ascontiguousarray(np.asarray(signals, np.float32)[:, :, 0])
    L = np.asarray(sig_length).astype(np.int64)
    k1 = np.stack([np.asarray(k1w, np.float32)[0, 0],
                   np.asarray(k1aw, np.float32)[0, 0]])  # [2, C]
    k1abT = np.ascontiguousarray(
        np.asarray(k1ab, np.float32).reshape(2, 128).T)  # [128, 2]
    k2w = np.asarray(k2w, np.float32)
    k3 = np.ascontiguousarray(np.asarray(k3w, np.float32)[0])
    Wf = np.asarray(Wf, np.float32); Wb = np.asarray(Wb, np.float32)
    bfp = _perm_bias(np.asarray(bf, np.float32))
    bbp = _perm_bias(np.asarray(bb, np.float32))
    Wd = np.asarray(Wd, np.float32); bd = np.asarray(bd, np.float32)

    Wxf = _perm_cols(Wf[:C]); Whf = _perm_cols(Wf[C:])
    Wxb = _perm_cols(Wb[:C]); Whb_ = _perm_cols(Wb[C:])

    # ---------------- launch 1: conv + zx ----------------
    sig_rev = np.ascontiguousarray(sig[:, ::-1])
    k2_flip = np.ascontiguousarray(k2w[::-1])
    sig_p = np.pad(sig, ((0, 0), (1, 1)))
    sig_rp = np.pad(sig_rev, ((0, 0), (1, 1)))
    in_maps = []
    for g in range(4):
        in_maps.append(dict(sig=_bf16(sig_p[8 * g:8 * g + 8]), k1=_bf16(k1),
                            k1abT=k1abT, k2=_bf16(k2w), k3=_bf16(k3),
                            wx=_bf16(Wxf)))
    for g in range(4):
        in_maps.append(dict(sig=_bf16(sig_rp[8 * g:8 * g + 8]), k1=_bf16(k1),
                            k1abT=k1abT, k2=_bf16(k2_flip), k3=_bf16(k3),
                            wx=_bf16(Wxb)))
    zx_list, _ = run_conv_zx(in_maps)

    # zx_f/zx_b: [32, T, 800] bf16 (bw rows are fully time-reversed)
    zx_f = np.concatenate([np.asarray(z) for z in zx_list[0:4]], axis=0)
    zx_b = np.concatenate([np.asarray(z) for z in zx_list[4:8]], axis=0)

    # length reset for bw: zero state entering scan step T-L by forcing
    # i/f gate logits to -30 at step T-L-1 (c_new ~ 0, h_new ~ 0).
    NEG = ml_dtypes.bfloat16(-30.0)
    for b in range(B):
        tr = T - int(L[b]) - 1
        if 0 <= tr < T:
            zx_b[b, tr, 0:H] = NEG
            zx_b[b, tr, 2 * H:3 * H] = NEG

    # per-stream zx assembly: chunk k covers steps [k*CH, (k+1)*CH) with
    # WARM warmup steps before; chunk 0's warmup is the reset pattern.
    reset_blk = np.zeros((B, WARM, G4), ml_dtypes.bfloat16)
    reset_blk[:, :, 0:H] = NEG
    reset_blk[:, :, 2 * H:3 * H] = NEG

    def stream_zx(zx_full, k):
        t0 = k * CH
        if t0 == 0:
            return np.concatenate([reset_blk, zx_full[:, 0:CH]], axis=1)
        return zx_full[:, t0 - WARM:t0 + CH]

    # ---------------- launch 2: recurrence ----------------
    id32 = np.eye(32, dtype=np.float32)
    in_maps2 = []
    for c in range(8):
        if c < 4:
            zxd, wh, bp = zx_f, Whf, bfp
        else:
            zxd, wh, bp = zx_b, Whb_, bbp
        k0 = 4 * (c % 4)
        zxin = np.stack([stream_zx(zxd, k0 + s) for s in range(4)], axis=0)
        in_maps2.append(dict(w0=_bf16(wh[0:128]), w1=_bf16(wh[128:200]),
                             bT=_bf16(bp[None, :]), id32=_bf16(id32),
                             zxin=zxin))
    hseqs, _ = run_lstm(in_maps2)

    # ---------------- host decode ----------------
    fw = np.empty((B, T, H), np.float32)
    bw_s = np.empty((B, T, H), np.float32)
    for c in range(8):
        hs = np.asarray(hseqs[c])
        dst = fw if c < 4 else bw_s
        for s in range(4):
            k = 4 * (c % 4) + s
            h = _unpack_hseqT(hs[s], NSTEP)[:, WARM:]
            dst[:, k * CH:(k + 1) * CH] = h
    bw = bw_s[:, ::-1, :]                                      # t = T-1-s
    bi = np.concatenate([fw, bw], axis=-1)                     # [32, T, 2H]
    logits = bi.reshape(-1, 2 * H) @ Wd + bd
    logits = logits.reshape(B, T, 5).astype(np.float32)
    tmask = np.arange(T)[None, :] >= L[:, None]
    logits[tmask] = bd
    return logits


if __name__ == "__main__":
    import jax, reference
    cpu = jax.devices("cpu")[0]
    with jax.default_device(cpu):
        inputs = {k: np.asarray(v) for k, v in reference.setup_inputs().items()}
        expected = np.asarray(jax.jit(reference.reference, backend="cpu")(
            **{k: jax.device_put(v, cpu) for k, v in inputs.items()}))
    actual = kernel(**inputs)
    err = np.abs(actual - expected).max() / (np.abs(expected).max() + 1e-9)
    print("Relative error:", err)


# revision 19
# speedup vs baseline: 1.0122x; 1.0053x over previous
"""Bidirectional-LSTM basecaller on 8 Trainium2 NeuronCores (self-contained).

Layout (HW time, concourse cost model): conv 0.273 ms + lstm 0.494 ms
= 0.767 ms total (baseline for this problem: 12.65 ms).

Launch 1 "conv" (8 cores, SPMD over batch x direction): conv front-end +
  zx = enc@Wx (cores 0-3: forward batch rows; 4-7: time-reversed rows with
  tap-flipped conv kernels -- exact for full reversal).  All matmul operands
  bf16 (1 cycle/row vs 4 for fp32).  ReLUs balanced across ACT and DVE
  (DVE tensor_scalar max / scalar_tensor_tensor fuses relu+residual-add);
  gate biases NOT added here (folded into the lstm's accumulation).
  zx gate cols [I|2J|F|O], J pre-doubled (tanh j = 2*sigmoid(2j)-1).

Launch 2 "lstm" (8 cores): time-chunked recurrence.  Each direction's
  T=2048 steps split into 16 chunks of 128 + 40 warmup steps (forget-gate
  state decay makes truncated history exact to ~4e-3); 4 chunks (streams)
  per core -> 168 serial steps instead of 2048.  State kept TRANSPOSED
  ([200, 32] packed as [128, 64] tiles: block A = rows 0:128 at cols 0:32,
  block B = rows 128:200 on partitions 0:72 at cols 32:64) so the
  recurrence needs no per-step transpose:
    - z^T via 32 small matmuls/step: per (gate, block) chunk, accumulate
      lhsT = zx ring slot (identity rhs injects zx^T), bias row (ones rhs),
      Wh[0:128] (rhs = hA), Wh[128:200] (rhs = hB);
    - ONE sigmoid over all 256 psum cols -> bf16 gates in SBUF;
    - DVE: p/2 = (sig2j - 0.5)*sigi; cf = c*sigf; c' = 2*(p/2) + cf
      (scalar_tensor_tensor fusions); ACT tanh; h = tanh(c')*sigo in one
      [128, 64] op (garbage B-rows 72:128 bounded + never read);
  Per-step latency ~2.94 us, ACT-engine-bound (sigmoid 398 + tanh 238 ns
  busy per stream-step; 4 streams x 636 = 2544 ns of the period).  hseq
  stores issue from the idle Pool queue (SP.SEQ is saturated by ring
  prefetch DMAs at ~2.9 us SEQ-hold each).  Length masking is folded into zx as i/f gate
  logits = -30 at the reset step (exact to ~1e-12), so steps have no mask
  ops.  h history is stored transposed and unpacked on host.

HW facts this build relies on: matmul start=True zeroes the WHOLE PSUM bank
  -> exactly one start per step's accumulation group (skip_group_check);
  lhsT/rhs/psum base partitions 0; bf16 operands for 1-cycle/row matmuls
  and 4x-mode DVE; zx ring = 3 groups of 8 steps, prefetched 1 group ahead
  (slot's last reader finished a full group earlier).

Host: shard prep, zx chunk/warmup assembly (chunk-0 warmup = reset
  pattern), gather, output reversal, valid-length masking, 400x5 decode.
"""
import numpy as np
from contextlib import ExitStack

import concourse.bass as bass
import concourse.bacc as bacc
import concourse.mybir as mybir
from concourse.tile import TileContext
from concourse.bass_utils import run_bass_kernel_spmd

B, T, H, C = 32, 2048, 200, 256
G4 = 4 * H  # 800
FP32 = mybir.dt.float32
BF16 = mybir.dt.bfloat16
SIG = mybir.ActivationFunctionType.Sigmoid
TANH = mybir.ActivationFunctionType.Tanh
RELU = mybir.ActivationFunctionType.Relu
MULT = mybir.AluOpType.mult
ADD = mybir.AluOpType.add
AMAX = mybir.AluOpType.max
SUB = mybir.AluOpType.subtract

CH = 128    # lstm chunk length (16 chunks per direction)
WARM = 40   # warmup steps per chunk
NSTEP = CH + WARM
HU = 8      # steps per hseq tile / ring group


# ----------------------------------------------------------------------------
# Launch 1: conv front-end + zx precompute. 8 (row, dir) pairs per core.
# ----------------------------------------------------------------------------
def build_conv_zx(n_rows=8, tchunk=256):
    nc = bacc.Bacc()
    sig = nc.declare_dram_parameter("sig", [n_rows, T + 2], BF16, isOutput=False)
    k1 = nc.declare_dram_parameter("k1", [2, C], BF16, isOutput=False)
    k1abT = nc.declare_dram_parameter("k1abT", [128, 2], FP32, isOutput=False)
    k2 = nc.declare_dram_parameter("k2", [3, C, C], BF16, isOutput=False)
    k3 = nc.declare_dram_parameter("k3", [C, C], BF16, isOutput=False)
    wx = nc.declare_dram_parameter("wx", [C, G4], BF16, isOutput=False)
    zx = nc.declare_dram_parameter("zx", [n_rows, T, G4], BF16, isOutput=True)

    n_tc = T // tchunk
    with TileContext(nc) as tc:
        with ExitStack() as ctx:
            wpool = ctx.enter_context(tc.tile_pool(name="w", bufs=1))
            spool = ctx.enter_context(tc.tile_pool(name="s", bufs=4))
            c1pool = ctx.enter_context(tc.tile_pool(name="c1", bufs=3))
            c2pool = ctx.enter_context(tc.tile_pool(name="c2", bufs=3))
            epool = ctx.enter_context(tc.tile_pool(name="enc", bufs=3))
            zpool = ctx.enter_context(tc.tile_pool(name="zs", bufs=4))
            ppool = ctx.enter_context(tc.tile_pool(name="ps", bufs=2, space="PSUM"))
            p2pool = ctx.enter_context(tc.tile_pool(name="ps2", bufs=2, space="PSUM"))

            k1_t = wpool.tile([1, 2 * C], BF16)
            k1b_t = wpool.tile([128, 2], FP32)
            k2_t = [wpool.tile([128, 3 * C], BF16, tag=f"k2_{cs}", name=f"k2_{cs}")
                    for cs in range(2)]
            k3_t = [wpool.tile([128, C], BF16, tag=f"k3_{cs}", name=f"k3_{cs}")
                    for cs in range(2)]
            wx_t = [wpool.tile([128, G4], BF16, tag=f"wx_{cs}", name=f"wx_{cs}")
                    for cs in range(2)]
            # weight loads spread across SP/Pool queues (SP-serial DMA
            # issue costs ~2-3us each at startup)
            nc.sync.dma_start(out=k1_t[:, 0:C], in_=k1[0:1, :])
            nc.gpsimd.dma_start(out=k1_t[:, C:2 * C], in_=k1[1:2, :])
            nc.sync.dma_start(out=k1b_t[:], in_=k1abT[:])
            for cs in range(2):
                eng = nc.gpsimd if cs else nc.sync
                eng.dma_start(
                    out=k2_t[cs][:].rearrange("p (k c) -> p k c", k=3),
                    in_=k2[:, cs * 128:(cs + 1) * 128, :].transpose([1, 0, 2]))
                eng.dma_start(out=k3_t[cs][:],
                              in_=k3[cs * 128:(cs + 1) * 128, :])
                eng.dma_start(out=wx_t[cs][:],
                              in_=wx[cs * 128:(cs + 1) * 128, :])

            TC2 = tchunk + 2
            for r in range(n_rows):
                srow = spool.tile([1, T + 2], BF16, tag="srow")
                nc.sync.dma_start(out=srow[:], in_=sig[r:r + 1, :])
                for ci in range(n_tc):
                    t0 = ci * tchunk
                    st = srow[:, t0:t0 + TC2]
                    c1t = c1pool.tile([128, 2 * TC2], BF16, tag="c1")
                    c1at = c1pool.tile([128, 2 * tchunk], BF16, tag="c1a")
                    for cs in range(2):
                        ps = ppool.tile([128, TC2], FP32, tag="pa", bufs=1)
                        nc.tensor.matmul(
                            ps[:], k1_t[:, cs * 128:(cs + 1) * 128], st[:],
                            start=True, stop=True)
                        nc.vector.tensor_scalar_max(
                            c1t[:, cs * TC2:(cs + 1) * TC2], ps[:], 0.0)
                        ps2 = ppool.tile([128, tchunk], FP32, tag="pb", bufs=1)
                        nc.tensor.matmul(
                            ps2[:], k1_t[:, C + cs * 128:C + (cs + 1) * 128],
                            st[:, 1:tchunk + 1], start=True, stop=True)
                        nc.scalar.activation(
                            c1at[:, cs * tchunk:(cs + 1) * tchunk], ps2[:],
                            RELU, bias=k1b_t[:, cs:cs + 1])
                    c2t = c2pool.tile([128, 2 * tchunk], BF16)
                    for co in range(2):
                        ps = p2pool.tile([128, tchunk], FP32, tag="pc")
                        first = True
                        for k in range(3):
                            for cs in range(2):
                                nc.tensor.matmul(
                                    ps[:],
                                    k2_t[cs][:, k * C + co * 128:
                                             k * C + (co + 1) * 128],
                                    c1t[:, cs * TC2 + k:cs * TC2 + k + tchunk],
                                    start=first, stop=(k == 2 and cs == 1))
                                first = False
                        nc.vector.tensor_scalar_max(
                            c2t[:, co * tchunk:(co + 1) * tchunk], ps[:], 0.0)
                    et = epool.tile([128, 2 * tchunk], BF16)
                    for co in range(2):
                        ps = p2pool.tile([128, tchunk], FP32, tag="pd")
                        for cs in range(2):
                            nc.tensor.matmul(
                                ps[:],
                                k3_t[cs][:, co * 128:(co + 1) * 128],
                                c2t[:, cs * tchunk:(cs + 1) * tchunk],
                                start=(cs == 0), stop=(cs == 1))
                        nc.vector.scalar_tensor_tensor(
                            et[:, co * tchunk:(co + 1) * tchunk], ps[:], 0.0,
                            c1at[:, co * tchunk:(co + 1) * tchunk], AMAX, ADD)
                    for tt in range(tchunk // 128):
                        zs = zpool.tile([128, G4], BF16)
                        for half in range(2):
                            ps = p2pool.tile([128, 400], FP32, tag="pe")
                            for cs in range(2):
                                nc.tensor.matmul(
                                    ps[:],
                                    et[:, cs * tchunk + tt * 128:
                                       cs * tchunk + tt * 128 + 128],
                                    wx_t[cs][:, half * 400:(half + 1) * 400],
                                    start=(cs == 0), stop=(cs == 1))
                            if half == 0:
                                nc.vector.tensor_copy(zs[:, 0:400], ps[:])
                            else:
                                nc.scalar.copy(zs[:, 400:800], ps[:])
                        nc.sync.dma_start(
                            out=zx[r, t0 + tt * 128:t0 + (tt + 1) * 128, :],
                            in_=zs[:])
    nc.compile()
    return nc


# ----------------------------------------------------------------------------
# Launch 2: chunked LSTM recurrence, transposed state, 2 streams per core.
# ----------------------------------------------------------------------------
def build_lstm2(n_streams=4, n_steps=NSTEP, U=24):
    """zxin[s]: [32, n_steps, 800] bf16, gate cols [I|2J|F|O], bias folded,
    length-reset encoded as i/f=-30 cols.  hseqT[s]: [128, n_steps*64] bf16,
    h(t) packed-transposed at col t*64 (cols 0:32 = h rows 0:128; cols 32:64
    partitions 0:72 = h rows 128:200)."""
    nc = bacc.Bacc()
    w0 = nc.declare_dram_parameter("w0", [128, G4], BF16, isOutput=False)
    w1 = nc.declare_dram_parameter("w1", [72, G4], BF16, isOutput=False)
    bT = nc.declare_dram_parameter("bT", [1, G4], BF16, isOutput=False)
    id32 = nc.declare_dram_parameter("id32", [32, 32], BF16, isOutput=False)
    zxin = nc.declare_dram_parameter("zxin", [n_streams, 32, n_steps, G4],
                                     BF16, isOutput=False)
    hseqT = nc.declare_dram_parameter("hseqT", [n_streams, 128, n_steps * 64],
                                      BF16, isOutput=True)

    n_grp = n_steps // HU
    assert n_steps % HU == 0 and U % HU == 0

    with TileContext(nc) as tc:
        with ExitStack() as ctx:
            wpool = ctx.enter_context(tc.tile_pool(name="w", bufs=1))
            rpool = ctx.enter_context(tc.tile_pool(name="ring", bufs=1))
            spool = ctx.enter_context(tc.tile_pool(name="st", bufs=1))
            hpool = ctx.enter_context(tc.tile_pool(name="hst", bufs=3))
            gpool = ctx.enter_context(tc.tile_pool(name="g", bufs=3))
            tpool = ctx.enter_context(tc.tile_pool(name="tmp", bufs=2))
            zpsp = ctx.enter_context(tc.tile_pool(name="zps", bufs=2,
                                                  space="PSUM"))

            w0t = wpool.tile([128, G4], BF16)
            w1t = wpool.tile([72, G4], BF16)
            i32t = wpool.tile([32, 32], BF16)
            bTt = wpool.tile([1, G4], BF16)
            ones32 = wpool.tile([1, 32], BF16)
            nc.sync.dma_start(out=w0t[:], in_=w0[:])
            nc.sync.dma_start(out=w1t[:], in_=w1[:])
            nc.sync.dma_start(out=i32t[:], in_=id32[:])
            nc.sync.dma_start(out=bTt[:], in_=bT[:])
            nc.vector.memset(ones32[:], 1.0)

            rings, cts, h0s = [], [], []
            for s in range(n_streams):
                ring = rpool.tile([32, U * G4], BF16, tag=f"ring{s}",
                                  name=f"ring{s}")
                ct = spool.tile([128, 64], BF16, tag=f"ct{s}", name=f"ct{s}")
                h0 = spool.tile([128, 64], BF16, tag=f"h0{s}", name=f"h0{s}")
                nc.vector.memset(ct[:], 0.0)
                nc.vector.memset(h0[:], 0.0)
                # preload ring group 0 only; rest prefetched 1 group ahead
                # (split across SP/Pool queues: serialized SP issue costs
                # ~3us per DMA at startup)
                eng = nc.gpsimd if s % 2 else nc.sync
                eng.dma_start(
                    out=ring[:, 0:HU * G4
                             ].rearrange("p (s g) -> p s g", s=HU),
                    in_=zxin[s, :, 0:HU, :])
                rings.append(ring)
                cts.append(ct)
                h0s.append(h0)

            # chunk table: (psum col, zx/w col, K-size)
            chunks = []
            for g4 in range(4):
                chunks.append((g4 * 64, g4 * 200, 128))           # block A
                chunks.append((g4 * 64 + 32, g4 * 200 + 128, 72))  # block B

            def step(s, u, hst, hprev):
                slot = (u % U) * G4
                zp = zpsp.tile([128, 256], FP32, tag=f"zp{s}")
                # start=True zeroes the WHOLE PSUM bank -> exactly one start
                # (first zx matmul); everything else accumulates in place.
                first = True
                for co, gc, csz in chunks:
                    nc.tensor.matmul(
                        zp[0:csz, co:co + 32],
                        rings[s][:, slot + gc:slot + gc + csz],
                        i32t[:], start=first, stop=False,
                        skip_group_check=True)
                    first = False
                for co, gc, csz in chunks:
                    nc.tensor.matmul(
                        zp[0:csz, co:co + 32], bTt[:, gc:gc + csz],
                        ones32[:], start=False, stop=False,
                        skip_group_check=True)
                hA = hprev[:, 0:32]
                hB = hprev[0:72, 32:64]
                for idx, (co, gc, csz) in enumerate(chunks):
                    nc.tensor.matmul(zp[0:csz, co:co + 32],
                                     w0t[:, gc:gc + csz], hA,
                                     start=False, stop=False,
                                     skip_group_check=True)
                    nc.tensor.matmul(zp[0:csz, co:co + 32],
                                     w1t[:, gc:gc + csz], hB,
                                     start=False, stop=(idx == 7),
                                     skip_group_check=True)
                g = gpool.tile([128, 256], BF16, tag=f"g{s}")
                nc.scalar.activation(g[:], zp[:], SIG)
                # p/2 = (sig(2j) - 0.5) * sig(i)  [tanh j = 2 sig(2j) - 1]
                ph = tpool.tile([128, 64], BF16, tag=f"ph{s}")
                nc.vector.scalar_tensor_tensor(ph[:], g[:, 64:128], 0.5,
                                               g[:, 0:64], SUB, MULT)
                cf = tpool.tile([128, 64], BF16, tag=f"cf{s}")
                nc.vector.tensor_mul(cf[:], cts[s][:], g[:, 128:192])
                # c = 2*(p/2) + cf
                nc.vector.scalar_tensor_tensor(cts[s][:], ph[:], 2.0,
                                               cf[:], MULT, ADD)
                th = tpool.tile([128, 64], BF16, tag=f"th{s}")
                nc.scalar.activation(th[:], cts[s][:], TANH)
                o = (u % HU) * 64
                # one op; partitions 72:128 of the B half are garbage but
                # bounded (psum zeroed by start=True) and never read
                nc.vector.tensor_mul(hst[:, o:o + 64], th[:, 0:64],
                                     g[:, 192:256])

            hsts = [None] * n_streams
            for grp in range(n_grp):
                cur = []
                for s in range(n_streams):
                    # prefetch ring group grp+1 into its slot (ring holds
                    # U//HU=3 groups; the slot's last reader was grp-2, a
                    # full group ago -> safe even if lhsT WAR is untracked)
                    pg = grp + 1
                    if pg < n_grp:
                        half = (pg % (U // HU)) * HU
                        nc.sync.dma_start(
                            out=rings[s][:, half * G4:(half + HU) * G4
                                         ].rearrange("p (s g) -> p s g", s=HU),
                            in_=zxin[s, :, pg * HU:(pg + 1) * HU, :])
                    hst = hpool.tile([128, HU * 64], BF16, tag=f"hst{s}",
                                     name=f"hst{s}")
                    cur.append(hst)
                for s in range(n_streams):
                    for k in range(HU):
                        u = grp * HU + k
                        if u == 0:
                            hprev = h0s[s][:]
                        elif k == 0:
                            hprev = hsts[s][:, (HU - 1) * 64:HU * 64]
                        else:
                            hprev = cur[s][:, (k - 1) * 64:k * 64]
                        step(s, u, cur[s][:], hprev)
                for s in range(n_streams):
                    # issue from the otherwise-idle Pool queue: SP.SEQ is
                    # saturated by ring prefetches (~2.9us hold per DMA)
                    nc.gpsimd.dma_start(
                        out=hseqT[s, :, grp * HU * 64:(grp + 1) * HU * 64],
                        in_=cur[s][:])
                    hsts[s] = cur[s]
    nc.compile()
    return nc


# ----------------------------------------------------------------------------
# host-side runners
# ----------------------------------------------------------------------------
_NC_CACHE = {}
LAUNCH_WALLS = {}


def run_conv_zx(in_maps, **kw):
    import time
    if "conv" not in _NC_CACHE:
        _NC_CACHE["conv"] = build_conv_zx()
    nc = _NC_CACHE["conv"]
    t0 = time.time()
    res = run_bass_kernel_spmd(nc, in_maps, list(range(len(in_maps))), **kw)
    out = [r["zx"] for r in res.results]
    LAUNCH_WALLS["conv"] = time.time() - t0
    return out, res


def run_lstm(in_maps, **kw):
    import time
    if "lstm" not in _NC_CACHE:
        _NC_CACHE["lstm"] = build_lstm2()
    nc = _NC_CACHE["lstm"]
    t0 = time.time()
    res = run_bass_kernel_spmd(nc, in_maps, list(range(len(in_maps))), **kw)
    out = [r["hseqT"] for r in res.results]
    LAUNCH_WALLS["lstm"] = time.time() - t0
    return out, res


def _bf16(x):
    import ml_dtypes
    return np.asarray(x).astype(ml_dtypes.bfloat16)


def _perm_cols(w):
    """reference gate order [i, j, f, o] -> [I | 2*J | F | O] (800 cols)."""
    i, j, f, o = (w[..., k * H:(k + 1) * H] for k in range(4))
    return np.concatenate([i, 2.0 * j, f, o], axis=-1)


def _perm_bias(b):
    i, j, f, o = (b[k * H:(k + 1) * H] for k in range(4))
    return np.concatenate([i, 2.0 * j, f + 1.0, o], axis=-1)


def _unpack_hseqT(arr, n_steps):
    """[128, n_steps*64] bf16 -> [32, n_steps, 200] fp32"""
    a = np.asarray(arr, np.float32).reshape(128, n_steps, 2, 32)
    out = np.empty((32, n_steps, 200), np.float32)
    out[:, :, 0:128] = a[:, :, 0, :].transpose(2, 1, 0)
    out[:, :, 128:200] = a[0:72, :, 1, :].transpose(2, 1, 0)
    return out


def kernel(signals, sig_length, k1w, k1aw, k1ab, k2w, k3w, Wf, bf, Wb, bb,
           Wd, bd):
    import ml_dtypes
    sig = np.ascontiguousarray(np.asarray(signals, np.float32)[:, :, 0])
    L = np.asarray(sig_length).astype(np.int64)
    k1 = np.stack([np.asarray(k1w, np.float32)[0, 0],
                   np.asarray(k1aw, np.float32)[0, 0]])  # [2, C]
    k1abT = np.ascontiguousarray(
        np.asarray(k1ab, np.float32).reshape(2, 128).T)  # [128, 2]
    k2w = np.asarray(k2w, np.float32)
    k3 = np.ascontiguousarray(np.asarray(k3w, np.float32)[0])
    Wf = np.asarray(Wf, np.float32); Wb = np.asarray(Wb, np.float32)
    bfp = _perm_bias(np.asarray(bf, np.float32))
    bbp = _perm_bias(np.asarray(bb, np.float32))
    Wd = np.asarray(Wd, np.float32); bd = np.asarray(bd, np.float32)

    Wxf = _perm_cols(Wf[:C]); Whf = _perm_cols(Wf[C:])
    Wxb = _perm_cols(Wb[:C]); Whb_ = _perm_cols(Wb[C:])

    # ---------------- launch 1: conv + zx ----------------
    sig_rev = np.ascontiguousarray(sig[:, ::-1])
    k2_flip = np.ascontiguousarray(k2w[::-1])
    sig_p = np.pad(sig, ((0, 0), (1, 1)))
    sig_rp = np.pad(sig_rev, ((0, 0), (1, 1)))
    in_maps = []
    for g in range(4):
        in_maps.append(dict(sig=_bf16(sig_p[8 * g:8 * g + 8]), k1=_bf16(k1),
                            k1abT=k1abT, k2=_bf16(k2w), k3=_bf16(k3),
                            wx=_bf16(Wxf)))
    for g in range(4):
        in_maps.append(dict(sig=_bf16(sig_rp[8 * g:8 * g + 8]), k1=_bf16(k1),
                            k1abT=k1abT, k2=_bf16(k2_flip), k3=_bf16(k3),
                            wx=_bf16(Wxb)))
    zx_list, _ = run_conv_zx(in_maps)

    # zx_f/zx_b: [32, T, 800] bf16 (bw rows are fully time-reversed)
    zx_f = np.concatenate([np.asarray(z) for z in zx_list[0:4]], axis=0)
    zx_b = np.concatenate([np.asarray(z) for z in zx_list[4:8]], axis=0)

    # length reset for bw: zero state entering scan step T-L by forcing
    # i/f gate logits to -30 at step T-L-1 (c_new ~ 0, h_new ~ 0).
    NEG = ml_dtypes.bfloat16(-30.0)
    for b in range(B):
        tr = T - int(L[b]) - 1
        if 0 <= tr < T:
            zx_b[b, tr, 0:H] = NEG
            zx_b[b, tr, 2 * H:3 * H] = NEG

    # per-stream zx assembly: chunk k covers steps [k*CH, (k+1)*CH) with
    # WARM warmup steps before; chunk 0's warmup is the reset pattern.
    reset_blk = np.zeros((B, WARM, G4), ml_dtypes.bfloat16)
    reset_blk[:, :, 0:H] = NEG
    reset_blk[:, :, 2 * H:3 * H] = NEG

    def stream_zx(zx_full, k):
        t0 = k * CH
        if t0 == 0:
            return np.concatenate([reset_blk, zx_full[:, 0:CH]], axis=1)
        return zx_full[:, t0 - WARM:t0 + CH]

    # ---------------- launch 2: recurrence ----------------
    id32 = np.eye(32, dtype=np.float32)
    in_maps2 = []
    for c in range(8):
        if c < 4:
            zxd, wh, bp = zx_f, Whf, bfp
        else:
            zxd, wh, bp = zx_b, Whb_, bbp
        k0 = 4 * (c % 4)
        zxin = np.stack([stream_zx(zxd, k0 + s) for s in range(4)], axis=0)
        in_maps2.append(dict(w0=_bf16(wh[0:128]), w1=_bf16(wh[128:200]),
                             bT=_bf16(bp[None, :]), id32=_bf16(id32),
                             zxin=zxin))
    hseqs, _ = run_lstm(in_maps2)

    # ---------------- host decode ----------------
    fw = np.empty((B, T, H), np.float32)
    bw_s = np.empty((B, T, H), np.float32)
    for c in range(8):
        hs = np.asarray(hseqs[c])
        dst = fw if c < 4 else bw_s
        for s in range(4):
            k = 4 * (c % 4) + s
            h = _unpack_hseqT(hs[s], NSTEP)[:, WARM:]
            dst[:, k * CH:(k + 1) * CH] = h
    bw = bw_s[:, ::-1, :]                                      # t = T-1-s
    bi = np.concatenate([fw, bw], axis=-1)                     # [32, T, 2H]
    logits = bi.reshape(-1, 2 * H) @ Wd + bd
    logits = logits.reshape(B, T, 5).astype(np.float32)
    tmask = np.arange(T)[None, :] >= L[:, None]
    logits[tmask] = bd
    return logits


if __name__ == "__main__":
    import jax, reference
    cpu = jax.devices("cpu")[0]
    with jax.default_device(cpu):
        inputs = {k: np.asarray(v) for k, v in reference.setup_inputs().items()}
        expected = np.asarray(jax.jit(reference.reference, backend="cpu")(
            **{k: jax.device_put(v, cpu) for k, v in inputs.items()}))
    actual = kernel(**inputs)
    err = np.abs(actual - expected).max() / (np.abs(expected).max() + 1e-9)
    print("Relative error:", err)


# revision 20
# speedup vs baseline: 1.0135x; 1.0013x over previous
"""Bidirectional-LSTM basecaller on 8 Trainium2 NeuronCores (self-contained).

Layout (HW time, concourse cost model): conv 0.273 ms + lstm 0.494 ms
= 0.767 ms total (baseline for this problem: 12.65 ms).

Launch 1 "conv" (8 cores, SPMD over batch x direction): conv front-end +
  zx = enc@Wx (cores 0-3: forward batch rows; 4-7: time-reversed rows with
  tap-flipped conv kernels -- exact for full reversal).  All matmul operands
  bf16 (1 cycle/row vs 4 for fp32).  ReLUs balanced across ACT and DVE
  (DVE tensor_scalar max / scalar_tensor_tensor fuses relu+residual-add);
  gate biases NOT added here (folded into the lstm's accumulation).
  zx gate cols [I|2J|F|O], J pre-doubled (tanh j = 2*sigmoid(2j)-1).

Launch 2 "lstm" (8 cores): time-chunked recurrence.  Each direction's
  T=2048 steps split into 16 chunks of 128 + 40 warmup steps (forget-gate
  state decay makes truncated history exact to ~4e-3); 4 chunks (streams)
  per core -> 168 serial steps instead of 2048.  State kept TRANSPOSED
  ([200, 32] packed as [128, 64] tiles: block A = rows 0:128 at cols 0:32,
  block B = rows 128:200 on partitions 0:72 at cols 32:64) so the
  recurrence needs no per-step transpose:
    - z^T via 32 small matmuls/step: per (gate, block) chunk, accumulate
      lhsT = zx ring slot (identity rhs injects zx^T), bias row (ones rhs),
      Wh[0:128] (rhs = hA), Wh[128:200] (rhs = hB);
    - ONE sigmoid over all 256 psum cols -> bf16 gates in SBUF;
    - DVE: p/2 = (sig2j - 0.5)*sigi; cf = c*sigf; c' = 2*(p/2) + cf
      (scalar_tensor_tensor fusions); ACT tanh; h = tanh(c')*sigo in one
      [128, 64] op (garbage B-rows 72:128 bounded + never read);
  Per-step latency ~2.94 us, ACT-engine-bound (sigmoid 398 + tanh 238 ns
  busy per stream-step; 4 streams x 636 = 2544 ns of the period).  hseq
  stores issue from the idle Pool queue (SP.SEQ is saturated by ring
  prefetch DMAs at ~2.9 us SEQ-hold each).  Length masking is folded into zx as i/f gate
  logits = -30 at the reset step (exact to ~1e-12), so steps have no mask
  ops.  h history is stored transposed and unpacked on host.

HW facts this build relies on: matmul start=True zeroes the WHOLE PSUM bank
  -> exactly one start per step's accumulation group (skip_group_check);
  lhsT/rhs/psum base partitions 0; bf16 operands for 1-cycle/row matmuls
  and 4x-mode DVE; zx ring = 3 groups of 8 steps, prefetched 1 group ahead
  (slot's last reader finished a full group earlier).

Host: shard prep, zx chunk/warmup assembly (chunk-0 warmup = reset
  pattern), gather, output reversal, valid-length masking, 400x5 decode.
"""
import numpy as np
from contextlib import ExitStack

import concourse.bass as bass
import concourse.bacc as bacc
import concourse.mybir as mybir
from concourse.tile import TileContext
from concourse.bass_utils import run_bass_kernel_spmd

B, T, H, C = 32, 2048, 200, 256
G4 = 4 * H  # 800
FP32 = mybir.dt.float32
BF16 = mybir.dt.bfloat16
SIG = mybir.ActivationFunctionType.Sigmoid
TANH = mybir.ActivationFunctionType.Tanh
RELU = mybir.ActivationFunctionType.Relu
MULT = mybir.AluOpType.mult
ADD = mybir.AluOpType.add
AMAX = mybir.AluOpType.max
SUB = mybir.AluOpType.subtract

CH = 128    # lstm chunk length (16 chunks per direction)
WARM = 40   # warmup steps per chunk
NSTEP = CH + WARM
HU = 8      # steps per hseq tile / ring group


# ----------------------------------------------------------------------------
# Launch 1: conv front-end + zx precompute. 8 (row, dir) pairs per core.
# ----------------------------------------------------------------------------
def build_conv_zx(n_rows=8, tchunk=256):
    nc = bacc.Bacc()
    sig = nc.declare_dram_parameter("sig", [n_rows, T + 2], BF16, isOutput=False)
    k1 = nc.declare_dram_parameter("k1", [2, C], BF16, isOutput=False)
    k1abT = nc.declare_dram_parameter("k1abT", [128, 2], FP32, isOutput=False)
    k2 = nc.declare_dram_parameter("k2", [3, C, C], BF16, isOutput=False)
    k3 = nc.declare_dram_parameter("k3", [C, C], BF16, isOutput=False)
    wx = nc.declare_dram_parameter("wx", [C, G4], BF16, isOutput=False)
    zx = nc.declare_dram_parameter("zx", [n_rows, T, G4], BF16, isOutput=True)

    n_tc = T // tchunk
    with TileContext(nc) as tc:
        with ExitStack() as ctx:
            wpool = ctx.enter_context(tc.tile_pool(name="w", bufs=1))
            spool = ctx.enter_context(tc.tile_pool(name="s", bufs=4))
            c1pool = ctx.enter_context(tc.tile_pool(name="c1", bufs=3))
            c2pool = ctx.enter_context(tc.tile_pool(name="c2", bufs=3))
            epool = ctx.enter_context(tc.tile_pool(name="enc", bufs=3))
            zpool = ctx.enter_context(tc.tile_pool(name="zs", bufs=4))
            ppool = ctx.enter_context(tc.tile_pool(name="ps", bufs=2, space="PSUM"))
            p2pool = ctx.enter_context(tc.tile_pool(name="ps2", bufs=2, space="PSUM"))

            k1_t = wpool.tile([1, 2 * C], BF16)
            k1b_t = wpool.tile([128, 2], FP32)
            k2_t = [wpool.tile([128, 3 * C], BF16, tag=f"k2_{cs}", name=f"k2_{cs}")
                    for cs in range(2)]
            k3_t = [wpool.tile([128, C], BF16, tag=f"k3_{cs}", name=f"k3_{cs}")
                    for cs in range(2)]
            wx_t = [wpool.tile([128, G4], BF16, tag=f"wx_{cs}", name=f"wx_{cs}")
                    for cs in range(2)]
            # weight loads spread across SP/Pool queues (SP-serial DMA
            # issue costs ~2-3us each at startup)
            nc.sync.dma_start(out=k1_t[:, 0:C], in_=k1[0:1, :])
            nc.gpsimd.dma_start(out=k1_t[:, C:2 * C], in_=k1[1:2, :])
            nc.sync.dma_start(out=k1b_t[:], in_=k1abT[:])
            for cs in range(2):
                eng = nc.gpsimd if cs else nc.sync
                eng.dma_start(
                    out=k2_t[cs][:].rearrange("p (k c) -> p k c", k=3),
                    in_=k2[:, cs * 128:(cs + 1) * 128, :].transpose([1, 0, 2]))
                eng.dma_start(out=k3_t[cs][:],
                              in_=k3[cs * 128:(cs + 1) * 128, :])
                eng.dma_start(out=wx_t[cs][:],
                              in_=wx[cs * 128:(cs + 1) * 128, :])

            TC2 = tchunk + 2
            for r in range(n_rows):
                srow = spool.tile([1, T + 2], BF16, tag="srow")
                nc.sync.dma_start(out=srow[:], in_=sig[r:r + 1, :])
                for ci in range(n_tc):
                    t0 = ci * tchunk
                    st = srow[:, t0:t0 + TC2]
                    c1t = c1pool.tile([128, 2 * TC2], BF16, tag="c1")
                    c1at = c1pool.tile([128, 2 * tchunk], BF16, tag="c1a")
                    for cs in range(2):
                        ps = ppool.tile([128, TC2], FP32, tag="pa", bufs=1)
                        nc.tensor.matmul(
                            ps[:], k1_t[:, cs * 128:(cs + 1) * 128], st[:],
                            start=True, stop=True)
                        nc.vector.tensor_scalar_max(
                            c1t[:, cs * TC2:(cs + 1) * TC2], ps[:], 0.0)
                        ps2 = ppool.tile([128, tchunk], FP32, tag="pb", bufs=1)
                        nc.tensor.matmul(
                            ps2[:], k1_t[:, C + cs * 128:C + (cs + 1) * 128],
                            st[:, 1:tchunk + 1], start=True, stop=True)
                        nc.scalar.activation(
                            c1at[:, cs * tchunk:(cs + 1) * tchunk], ps2[:],
                            RELU, bias=k1b_t[:, cs:cs + 1])
                    c2t = c2pool.tile([128, 2 * tchunk], BF16)
                    for co in range(2):
                        ps = p2pool.tile([128, tchunk], FP32, tag="pc")
                        first = True
                        for k in range(3):
                            for cs in range(2):
                                nc.tensor.matmul(
                                    ps[:],
                                    k2_t[cs][:, k * C + co * 128:
                                             k * C + (co + 1) * 128],
                                    c1t[:, cs * TC2 + k:cs * TC2 + k + tchunk],
                                    start=first, stop=(k == 2 and cs == 1))
                                first = False
                        nc.vector.tensor_scalar_max(
                            c2t[:, co * tchunk:(co + 1) * tchunk], ps[:], 0.0)
                    et = epool.tile([128, 2 * tchunk], BF16)
                    for co in range(2):
                        ps = p2pool.tile([128, tchunk], FP32, tag="pd")
                        for cs in range(2):
                            nc.tensor.matmul(
                                ps[:],
                                k3_t[cs][:, co * 128:(co + 1) * 128],
                                c2t[:, cs * tchunk:(cs + 1) * tchunk],
                                start=(cs == 0), stop=(cs == 1))
                        nc.vector.scalar_tensor_tensor(
                            et[:, co * tchunk:(co + 1) * tchunk], ps[:], 0.0,
                            c1at[:, co * tchunk:(co + 1) * tchunk], AMAX, ADD)
                    for tt in range(tchunk // 128):
                        zs = zpool.tile([128, G4], BF16)
                        for half in range(2):
                            ps = p2pool.tile([128, 400], FP32, tag="pe")
                            for cs in range(2):
                                nc.tensor.matmul(
                                    ps[:],
                                    et[:, cs * tchunk + tt * 128:
                                       cs * tchunk + tt * 128 + 128],
                                    wx_t[cs][:, half * 400:(half + 1) * 400],
                                    start=(cs == 0), stop=(cs == 1))
                            if half == 0:
                                nc.vector.tensor_copy(zs[:, 0:400], ps[:])
                            else:
                                nc.scalar.copy(zs[:, 400:800], ps[:])
                        nc.sync.dma_start(
                            out=zx[r, t0 + tt * 128:t0 + (tt + 1) * 128, :],
                            in_=zs[:])
    nc.compile()
    return nc


# ----------------------------------------------------------------------------
# Launch 2: chunked LSTM recurrence, transposed state, 2 streams per core.
# ----------------------------------------------------------------------------
def build_lstm2(n_streams=4, n_steps=NSTEP, U=24):
    """zxin[s]: [32, n_steps, 800] bf16, gate cols [I|2J|F|O], bias folded,
    length-reset encoded as i/f=-30 cols.  hseqT[s]: [128, n_steps*64] bf16,
    h(t) packed-transposed at col t*64 (cols 0:32 = h rows 0:128; cols 32:64
    partitions 0:72 = h rows 128:200)."""
    nc = bacc.Bacc()
    w0 = nc.declare_dram_parameter("w0", [128, G4], BF16, isOutput=False)
    w1 = nc.declare_dram_parameter("w1", [72, G4], BF16, isOutput=False)
    bT = nc.declare_dram_parameter("bT", [1, G4], BF16, isOutput=False)
    id32 = nc.declare_dram_parameter("id32", [32, 32], BF16, isOutput=False)
    zxin = nc.declare_dram_parameter("zxin", [n_streams, 32, n_steps, G4],
                                     BF16, isOutput=False)
    hseqT = nc.declare_dram_parameter("hseqT", [n_streams, 128, n_steps * 64],
                                      BF16, isOutput=True)

    n_grp = n_steps // HU
    assert n_steps % HU == 0 and U % HU == 0

    with TileContext(nc) as tc:
        with ExitStack() as ctx:
            wpool = ctx.enter_context(tc.tile_pool(name="w", bufs=1))
            rpool = ctx.enter_context(tc.tile_pool(name="ring", bufs=1))
            spool = ctx.enter_context(tc.tile_pool(name="st", bufs=1))
            hpool = ctx.enter_context(tc.tile_pool(name="hst", bufs=3))
            gpool = ctx.enter_context(tc.tile_pool(name="g", bufs=3))
            tpool = ctx.enter_context(tc.tile_pool(name="tmp", bufs=2))
            zpsp = ctx.enter_context(tc.tile_pool(name="zps", bufs=2,
                                                  space="PSUM"))

            w0t = wpool.tile([128, G4], BF16)
            w1t = wpool.tile([72, G4], BF16)
            i32t = wpool.tile([32, 32], BF16)
            bTt = wpool.tile([1, G4], BF16)
            ones32 = wpool.tile([1, 32], BF16)
            nc.sync.dma_start(out=w0t[:], in_=w0[:])
            nc.gpsimd.dma_start(out=w1t[:], in_=w1[:])
            nc.gpsimd.dma_start(out=i32t[:], in_=id32[:])
            nc.sync.dma_start(out=bTt[:], in_=bT[:])
            nc.vector.memset(ones32[:], 1.0)

            rings, cts, h0s = [], [], []
            for s in range(n_streams):
                ring = rpool.tile([32, U * G4], BF16, tag=f"ring{s}",
                                  name=f"ring{s}")
                ct = spool.tile([128, 64], BF16, tag=f"ct{s}", name=f"ct{s}")
                h0 = spool.tile([128, 64], BF16, tag=f"h0{s}", name=f"h0{s}")
                nc.vector.memset(ct[:], 0.0)
                nc.vector.memset(h0[:], 0.0)
                # preload ring group 0 only; rest prefetched 1 group ahead
                # (split across SP/Pool queues: serialized SP issue costs
                # ~3us per DMA at startup)
                eng = nc.gpsimd if s % 2 else nc.sync
                eng.dma_start(
                    out=ring[:, 0:HU * G4
                             ].rearrange("p (s g) -> p s g", s=HU),
                    in_=zxin[s, :, 0:HU, :])
                rings.append(ring)
                cts.append(ct)
                h0s.append(h0)

            # chunk table: (psum col, zx/w col, K-size)
            chunks = []
            for g4 in range(4):
                chunks.append((g4 * 64, g4 * 200, 128))           # block A
                chunks.append((g4 * 64 + 32, g4 * 200 + 128, 72))  # block B

            def step(s, u, hst, hprev):
                slot = (u % U) * G4
                zp = zpsp.tile([128, 256], FP32, tag=f"zp{s}")
                # start=True zeroes the WHOLE PSUM bank -> exactly one start
                # (first zx matmul); everything else accumulates in place.
                first = True
                for co, gc, csz in chunks:
                    nc.tensor.matmul(
                        zp[0:csz, co:co + 32],
                        rings[s][:, slot + gc:slot + gc + csz],
                        i32t[:], start=first, stop=False,
                        skip_group_check=True)
                    first = False
                for co, gc, csz in chunks:
                    nc.tensor.matmul(
                        zp[0:csz, co:co + 32], bTt[:, gc:gc + csz],
                        ones32[:], start=False, stop=False,
                        skip_group_check=True)
                hA = hprev[:, 0:32]
                hB = hprev[0:72, 32:64]
                for idx, (co, gc, csz) in enumerate(chunks):
                    nc.tensor.matmul(zp[0:csz, co:co + 32],
                                     w0t[:, gc:gc + csz], hA,
                                     start=False, stop=False,
                                     skip_group_check=True)
                    nc.tensor.matmul(zp[0:csz, co:co + 32],
                                     w1t[:, gc:gc + csz], hB,
                                     start=False, stop=(idx == 7),
                                     skip_group_check=True)
                g = gpool.tile([128, 256], BF16, tag=f"g{s}")
                nc.scalar.activation(g[:], zp[:], SIG)
                # p/2 = (sig(2j) - 0.5) * sig(i)  [tanh j = 2 sig(2j) - 1]
                ph = tpool.tile([128, 64], BF16, tag=f"ph{s}")
                nc.vector.scalar_tensor_tensor(ph[:], g[:, 64:128], 0.5,
                                               g[:, 0:64], SUB, MULT)
                cf = tpool.tile([128, 64], BF16, tag=f"cf{s}")
                nc.vector.tensor_mul(cf[:], cts[s][:], g[:, 128:192])
                # c = 2*(p/2) + cf
                nc.vector.scalar_tensor_tensor(cts[s][:], ph[:], 2.0,
                                               cf[:], MULT, ADD)
                th = tpool.tile([128, 64], BF16, tag=f"th{s}")
                nc.scalar.activation(th[:], cts[s][:], TANH)
                o = (u % HU) * 64
                # one op; partitions 72:128 of the B half are garbage but
                # bounded (psum zeroed by start=True) and never read
                nc.vector.tensor_mul(hst[:, o:o + 64], th[:, 0:64],
                                     g[:, 192:256])

            hsts = [None] * n_streams
            for grp in range(n_grp):
                cur = []
                for s in range(n_streams):
                    # prefetch ring group grp+1 into its slot (ring holds
                    # U//HU=3 groups; the slot's last reader was grp-2, a
                    # full group ago -> safe even if lhsT WAR is untracked)
                    pg = grp + 1
                    if pg < n_grp:
                        half = (pg % (U // HU)) * HU
                        nc.sync.dma_start(
                            out=rings[s][:, half * G4:(half + HU) * G4
                                         ].rearrange("p (s g) -> p s g", s=HU),
                            in_=zxin[s, :, pg * HU:(pg + 1) * HU, :])
                    hst = hpool.tile([128, HU * 64], BF16, tag=f"hst{s}",
                                     name=f"hst{s}")
                    cur.append(hst)
                for s in range(n_streams):
                    for k in range(HU):
                        u = grp * HU + k
                        if u == 0:
                            hprev = h0s[s][:]
                        elif k == 0:
                            hprev = hsts[s][:, (HU - 1) * 64:HU * 64]
                        else:
                            hprev = cur[s][:, (k - 1) * 64:k * 64]
                        step(s, u, cur[s][:], hprev)
                for s in range(n_streams):
                    # issue from the otherwise-idle Pool queue: SP.SEQ is
                    # saturated by ring prefetches (~2.9us hold per DMA)
                    nc.gpsimd.dma_start(
                        out=hseqT[s, :, grp * HU * 64:(grp + 1) * HU * 64],
                        in_=cur[s][:])
                    hsts[s] = cur[s]
    nc.compile()
    return nc


# ----------------------------------------------------------------------------
# host-side runners
# ----------------------------------------------------------------------------
_NC_CACHE = {}
LAUNCH_WALLS = {}


def run_conv_zx(in_maps, **kw):
    import time
    if "conv" not in _NC_CACHE:
        _NC_CACHE["conv"] = build_conv_zx()
    nc = _NC_CACHE["conv"]
    t0 = time.time()
    res = run_bass_kernel_spmd(nc, in_maps, list(range(len(in_maps))), **kw)
    out = [r["zx"] for r in res.results]
    LAUNCH_WALLS["conv"] = time.time() - t0
    return out, res


def run_lstm(in_maps, **kw):
    import time
    if "lstm" not in _NC_CACHE:
        _NC_CACHE["lstm"] = build_lstm2()
    nc = _NC_CACHE["lstm"]
    t0 = time.time()
    res = run_bass_kernel_spmd(nc, in_maps, list(range(len(in_maps))), **kw)
    out = [r["hseqT"] for r in res.results]
    LAUNCH_WALLS["lstm"] = time.time() - t0
    return out, res


def _bf16(x):
    import ml_dtypes
    return np.asarray(x).astype(ml_dtypes.bfloat16)


def _perm_cols(w):
    """reference gate order [i, j, f, o] -> [I | 2*J | F | O] (800 cols)."""
    i, j, f, o = (w[..., k * H:(k + 1) * H] for k in range(4))
    return np.concatenate([i, 2.0 * j, f, o], axis=-1)


def _perm_bias(b):
    i, j, f, o = (b[k * H:(k + 1) * H] for k in range(4))
    return np.concatenate([i, 2.0 * j, f + 1.0, o], axis=-1)


def _unpack_hseqT(arr, n_steps):
    """[128, n_steps*64] bf16 -> [32, n_steps, 200] fp32"""
    a = np.asarray(arr, np.float32).reshape(128, n_steps, 2, 32)
    out = np.empty((32, n_steps, 200), np.float32)
    out[:, :, 0:128] = a[:, :, 0, :].transpose(2, 1, 0)
    out[:, :, 128:200] = a[0:72, :, 1, :].transpose(2, 1, 0)
    return out


def kernel(signals, sig_length, k1w, k1aw, k1ab, k2w, k3w, Wf, bf, Wb, bb,
           Wd, bd):
    import ml_dtypes
    sig = np.ascontiguousarray(np.asarray(signals, np.float32)[:, :, 0])
    L = np.asarray(sig_length).astype(np.int64)
    k1 = np.stack([np.asarray(k1w, np.float32)[0, 0],
                   np.asarray(k1aw, np.float32)[0, 0]])  # [2, C]
    k1abT = np.ascontiguousarray(
        np.asarray(k1ab, np.float32).reshape(2, 128).T)  # [128, 2]
    k2w = np.asarray(k2w, np.float32)
    k3 = np.ascontiguousarray(np.asarray(k3w, np.float32)[0])
    Wf = np.asarray(Wf, np.float32); Wb = np.asarray(Wb, np.float32)
    bfp = _perm_bias(np.asarray(bf, np.float32))
    bbp = _perm_bias(np.asarray(bb, np.float32))
    Wd = np.asarray(Wd, np.float32); bd = np.asarray(bd, np.float32)

    Wxf = _perm_cols(Wf[:C]); Whf = _perm_cols(Wf[C:])
    Wxb = _perm_cols(Wb[:C]); Whb_ = _perm_cols(Wb[C:])

    # ---------------- launch 1: conv + zx ----------------
    sig_rev = np.ascontiguousarray(sig[:, ::-1])
    k2_flip = np.ascontiguousarray(k2w[::-1])
    sig_p = np.pad(sig, ((0, 0), (1, 1)))
    sig_rp = np.pad(sig_rev, ((0, 0), (1, 1)))
    in_maps = []
    for g in range(4):
        in_maps.append(dict(sig=_bf16(sig_p[8 * g:8 * g + 8]), k1=_bf16(k1),
                            k1abT=k1abT, k2=_bf16(k2w), k3=_bf16(k3),
                            wx=_bf16(Wxf)))
    for g in range(4):
        in_maps.append(dict(sig=_bf16(sig_rp[8 * g:8 * g + 8]), k1=_bf16(k1),
                            k1abT=k1abT, k2=_bf16(k2_flip), k3=_bf16(k3),
                            wx=_bf16(Wxb)))
    zx_list, _ = run_conv_zx(in_maps)

    # zx_f/zx_b: [32, T, 800] bf16 (bw rows are fully time-reversed)
    zx_f = np.concatenate([np.asarray(z) for z in zx_list[0:4]], axis=0)
    zx_b = np.concatenate([np.asarray(z) for z in zx_list[4:8]], axis=0)

    # length reset for bw: zero state entering scan step T-L by forcing
    # i/f gate logits to -30 at step T-L-1 (c_new ~ 0, h_new ~ 0).
    NEG = ml_dtypes.bfloat16(-30.0)
    for b in range(B):
        tr = T - int(L[b]) - 1
        if 0 <= tr < T:
            zx_b[b, tr, 0:H] = NEG
            zx_b[b, tr, 2 * H:3 * H] = NEG

    # per-stream zx assembly: chunk k covers steps [k*CH, (k+1)*CH) with
    # WARM warmup steps before; chunk 0's warmup is the reset pattern.
    reset_blk = np.zeros((B, WARM, G4), ml_dtypes.bfloat16)
    reset_blk[:, :, 0:H] = NEG
    reset_blk[:, :, 2 * H:3 * H] = NEG

    def stream_zx(zx_full, k):
        t0 = k * CH
        if t0 == 0:
            return np.concatenate([reset_blk, zx_full[:, 0:CH]], axis=1)
        return zx_full[:, t0 - WARM:t0 + CH]

    # ---------------- launch 2: recurrence ----------------
    id32 = np.eye(32, dtype=np.float32)
    in_maps2 = []
    for c in range(8):
        if c < 4:
            zxd, wh, bp = zx_f, Whf, bfp
        else:
            zxd, wh, bp = zx_b, Whb_, bbp
        k0 = 4 * (c % 4)
        zxin = np.stack([stream_zx(zxd, k0 + s) for s in range(4)], axis=0)
        in_maps2.append(dict(w0=_bf16(wh[0:128]), w1=_bf16(wh[128:200]),
                             bT=_bf16(bp[None, :]), id32=_bf16(id32),
                             zxin=zxin))
    hseqs, _ = run_lstm(in_maps2)

    # ---------------- host decode ----------------
    fw = np.empty((B, T, H), np.float32)
    bw_s = np.empty((B, T, H), np.float32)
    for c in range(8):
        hs = np.asarray(hseqs[c])
        dst = fw if c < 4 else bw_s
        for s in range(4):
            k = 4 * (c % 4) + s
            h = _unpack_hseqT(hs[s], NSTEP)[:, WARM:]
            dst[:, k * CH:(k + 1) * CH] = h
    bw = bw_s[:, ::-1, :]                                      # t = T-1-s
    bi = np.concatenate([fw, bw], axis=-1)                     # [32, T, 2H]
    logits = bi.reshape(-1, 2 * H) @ Wd + bd
    logits = logits.reshape(B, T, 5).astype(np.float32)
    tmask = np.arange(T)[None, :] >= L[:, None]
    logits[tmask] = bd
    return logits


if __name__ == "__main__":
    import jax, reference
    cpu = jax.devices("cpu")[0]
    with jax.default_device(cpu):
        inputs = {k: np.asarray(v) for k, v in reference.setup_inputs().items()}
        expected = np.asarray(jax.jit(reference.reference, backend="cpu")(
            **{k: jax.device_put(v, cpu) for k, v in inputs.items()}))
    actual = kernel(**inputs)
    err = np.abs(actual - expected).max() / (np.abs(expected).max() + 1e-9)
    print("Relative error:", err)
